# revision 12
# baseline (speedup 1.0000x reference)
"""Cross multi-head attention TRN2 kernel (8-core SPMD, head-sharded), v2.

Strategy (tensor parallel over heads, zero communication):
  - 16 heads / 8 cores -> 2 heads per core. Core c computes output columns
    [128*c, 128*(c+1)) of the [4096, 1024] output; host concatenates.
  - Host pre-transposes and PRE-TILES q/embed into [tile, P, chunks] bf16
    layouts so every input DMA is contiguous per partition.
  - Scores are computed transposed (S^T[k, q] = K.Q^T, scale folded into Wq).
    The two heads per core live on partition halves 0-63 / 64-127, so their
    K=64 score matmuls row-tile and overlap ~2x on the PE.
  - The exp stream is 128 uniform groups of 2 score slots (one kc, both
    heads) x [P, 2, 512] fp32 PSUM, double buffered (4 banks); ACT is the
    near-critical engine (~130us) and the PE (~140us) paces the kernel.
  - All other PE work (attn@V 2-kc chunks, projections, V-proj, ctx
    transposes) is a task list with (gate, deadline) bounds popped between
    exp groups; deadlines are tight so work never bursts at block edges.
  - Softmax denominator via a ones-column appended to V (attn@V also
    produces row-sums); ctx'^T is PE-transposed back to [q, d] (bf16),
    normalized per-partition (DVE reciprocal+mul), DMA'd out per block.
"""

import numpy as np
import ml_dtypes

import concourse.bass as bass
import concourse.bacc as bacc
import concourse.mybir as mybir
import concourse.tile as tile
from concourse.bass_utils import run_bass_kernel_spmd
from concourse.masks import make_identity

# ---- problem dims (hardcoded; kernel.py must be self-contained) ----
B, S, E = 2, 2048, 1024
NHEAD, HD = 16, 64
NCORES = 8
HPC = NHEAD // NCORES          # heads per core = 2
DPC = HPC * HD                 # projection out-dims per core = 128
ROWS = B * S                   # 4096
P = 128                        # SBUF partitions
NFREE = 512                    # matmul moving free dim (one PSUM bank fp32)
EC = E // P                    # 8 contraction chunks
KC = S // P                    # 16 key chunks per batch
QC = S // NFREE                # 4 query chunks per batch
RC_B = S // NFREE              # 4 projection row-chunks per batch
TPB = NFREE // P               # 4 transpose chunks per block
NSLOT = HPC * KC               # 32 score slots per (b,qc) block
NG = KC                        # 16 exp groups per block (2 slots each)
NB = B * QC                    # 8 blocks
NGT = NB * NG                  # 128 groups total
SCALE = 1.0 / np.sqrt(HD)      # 0.125, folded into Wq/bq on host

F32 = mybir.dt.float32
BF16 = mybir.dt.bfloat16
AF = mybir.ActivationFunctionType

_CACHED_NC = {}
LAST_RESULTS = None            # test.py reads exec_time_ns / profile from here

ORDER = [(0, 0), (0, 1), (0, 2), (0, 3), (1, 0), (1, 1), (1, 2), (1, 3)]


def _tile_inputs(mat_rows_e: np.ndarray) -> np.ndarray:
    """[ROWS, E] f32 -> pre-tiled bf16 [P, B*RC_B, EC*NFREE]: slice
    [:, b*RC_B+r, :] is one projection row-chunk, 8KB contiguous per
    partition, laid out [c, n] with E-index = c*128 + p, col = b*S +
    r*512 + n."""
    t = np.ascontiguousarray(mat_rows_e.T).astype(ml_dtypes.bfloat16)
    a = t.reshape(EC, P, B, RC_B, NFREE)            # [c, p, b, r, n]
    a = a.transpose(1, 2, 3, 0, 4)                  # [p, b, r, c, n]
    return np.ascontiguousarray(a.reshape(P, B * RC_B, EC * NFREE))


def _tile_w(wT: np.ndarray) -> np.ndarray:
    """[E, DPC] bf16 weight -> [P, EC*DPC]: contiguous per partition."""
    a = wT.reshape(EC, P, DPC).transpose(1, 0, 2)
    return np.ascontiguousarray(a.reshape(P, EC * DPC))


class _Task:
    """Filler work item: gate = earliest group index after whose exp it may
    be emitted; deadline = group whose SCORES it must precede (forced-pop
    at G >= deadline-2); cost = PE-ns estimate for pacing."""

    __slots__ = ("gate", "deadline", "cost", "fn")

    def __init__(self, gate, deadline, cost, fn):
        self.gate = gate
        self.deadline = deadline
        self.cost = cost
        self.fn = fn


def _build_nc(with_bias: bool) -> bass.Bass:
    nc = bacc.Bacc(
        "TRN2",
        target_bir_lowering=False,
        debug=False,
        num_devices=NCORES,
    )

    qTt = nc.declare_dram_parameter("qTt", [P, B * RC_B, EC * NFREE], BF16, isOutput=False)
    eTt = nc.declare_dram_parameter("eTt", [P, B * RC_B, EC * NFREE], BF16, isOutput=False)
    WqT = nc.declare_dram_parameter("WqT", [P, EC * DPC], BF16, isOutput=False)
    WkT = nc.declare_dram_parameter("WkT", [P, EC * DPC], BF16, isOutput=False)
    WvT = nc.declare_dram_parameter("WvT", [P, EC * DPC], BF16, isOutput=False)
    bqs = nc.declare_dram_parameter("bqs", [DPC], BF16, isOutput=False)
    bkp = nc.declare_dram_parameter("bkp", [DPC], BF16, isOutput=False)
    bvp = nc.declare_dram_parameter("bvp", [DPC], BF16, isOutput=False)
    out = nc.declare_dram_parameter("out", [ROWS, DPC], F32, isOutput=True)

    with tile.TileContext(nc) as tc:
        with (
            tc.tile_pool(name="consts", bufs=1) as consts,
            tc.tile_pool(name="wpool", bufs=1) as wpool,
            tc.tile_pool(name="resid", bufs=1) as resid,
            tc.tile_pool(name="esrc", bufs=1) as esrc,
            tc.tile_pool(name="qsrc", bufs=1) as qsrc,
            tc.tile_pool(name="prp", bufs=2) as prp,
            tc.tile_pool(name="misc", bufs=2) as misc,
            tc.tile_pool(name="otp", bufs=4) as otp,
            # PSUM banks: sp 2x2 + ctx 2 + ps 1 + tp 1 = 8
            tc.tile_pool(name="spp", bufs=2, space="PSUM") as spp,
            tc.tile_pool(name="pctx", bufs=2, space="PSUM") as pctx,
            tc.tile_pool(name="psmall", bufs=1, space="PSUM") as psmall,
        ):
            # ---------- weights first (first c-chunks split out so the
            # very first projection matmuls can start early) ----
            wk_sb = wpool.tile([P, EC, DPC], BF16, name="wk_sb")
            nc.sync.dma_start(
                wk_sb[:, 0:2], WkT.ap()[:, : 2 * DPC].rearrange("p (c d) -> p c d", c=2)
            )
            wq_sb = wpool.tile([P, EC, DPC], BF16, name="wq_sb")
            nc.scalar.dma_start(
                wq_sb[:, 0:2], WqT.ap()[:, : 2 * DPC].rearrange("p (c d) -> p c d", c=2)
            )
            nc.sync.dma_start(
                wk_sb[:, 2:], WkT.ap()[:, 2 * DPC :].rearrange("p (c d) -> p c d", c=EC - 2)
            )
            nc.scalar.dma_start(
                wq_sb[:, 2:], WqT.ap()[:, 2 * DPC :].rearrange("p (c d) -> p c d", c=EC - 2)
            )

            # ---------- source DMAs (chunked along E-contraction) --------
            src_chunks = {}

            def dma_src(which, b, r, lo, hi, eng):
                dram = qTt if which == "q" else eTt
                tag = f"{which}{hi - lo}"
                tl = (qsrc if which == "q" else esrc).tile(
                    [P, hi - lo, NFREE], BF16, tag=tag, bufs=(4 if hi - lo == 2 else 8),
                    name=f"{which}{b}{r}c{lo}",
                )
                eng.dma_start(
                    tl,
                    dram.ap()[:, b * RC_B + r, lo * NFREE : hi * NFREE].rearrange(
                        "p (c n) -> p c n", c=hi - lo
                    ),
                )
                src_chunks.setdefault((which, b, r), []).append((tl, lo, hi))

            def sl(which, b, r, c):
                for tl, lo, hi in src_chunks[(which, b, r)]:
                    if lo <= c < hi:
                        return tl[:, c - lo]
                raise KeyError((which, b, r, c))

            # startup-critical quarters: e00 on sync+scalar, q00 on
            # gpsimd+scalar (3 DMA-capable rings: sync, scalar, gpsimd)
            dma_src("e", 0, 0, 0, 2, nc.sync)
            dma_src("e", 0, 0, 4, 6, nc.scalar)
            dma_src("q", 0, 0, 0, 2, nc.gpsimd)
            dma_src("e", 0, 0, 2, 4, nc.sync)
            dma_src("e", 0, 0, 6, 8, nc.scalar)
            dma_src("q", 0, 0, 2, 4, nc.gpsimd)
            dma_src("q", 0, 0, 4, 6, nc.gpsimd)
            dma_src("q", 0, 0, 6, 8, nc.scalar)
            # wv + biases on gpsimd (needed by vproj from G0ish — must be
            # queued ahead of the bulky q01-03 halves)
            wv_sb = wpool.tile([P, EC, DPC], BF16, name="wv_sb")
            nc.gpsimd.dma_start(wv_sb, WvT.ap().rearrange("p (c d) -> p c d", c=EC))
            bq_sb = wpool.tile([1, DPC], BF16, name="bq_sb")
            nc.gpsimd.dma_start(bq_sb, bqs.ap()[None, :])
            bk_sb = wpool.tile([1, DPC], BF16, name="bk_sb")
            nc.gpsimd.dma_start(bk_sb, bkp.ap()[None, :])
            bv_sb = wpool.tile([1, DPC], BF16, name="bv_sb")
            nc.gpsimd.dma_start(bv_sb, bvp.ap()[None, :])
            # rest of b0 as halves; the first (c0-3) half of each e-tile
            # goes on the fast scalar ring so kproj can start per-tile early
            for r in (1, 2, 3):
                dma_src("e", 0, r, 0, 4, nc.scalar)
                dma_src("e", 0, r, 4, 8, nc.sync)
            for r in (1, 2, 3):
                dma_src("q", 0, r, 0, 4, nc.gpsimd)
                dma_src("q", 0, r, 4, 8, nc.gpsimd)

            # ---------- constants ----------
            ones_row = consts.tile([1, NFREE], BF16)
            nc.vector.memset(ones_row, 1.0)
            ident_bf = consts.tile([P, P], BF16)
            make_identity(nc, ident_bf)
            # warm the ACT exp table (after scalar's DMA issues)
            warm = consts.tile([1, 1], BF16)
            nc.scalar.activation(warm, ones_row[:, 0:1], AF.Exp)
            # warm the PE HAM (flip to 2.4 GHz) during the initial DMA wait:
            # a chain of dependency-free tiny matmuls spanning ~10us so the
            # real startup projections run at full clock
            wmp = psmall.tile([P, 32], F32, tag="ps", name="wmp")
            for _ in range(130):
                nc.tensor.matmul(
                    wmp, lhsT=ident_bf, rhs=ident_bf[:, 0:32], start=True, stop=True
                )

            # ---------- residents (per batch) ----------
            qt_sb = []
            kt_sb = []
            v_sb = []
            for b in range(B):
                qt = resid.tile([P, S], BF16, name=f"qt{b}")
                kt = resid.tile([P, S], BF16, name=f"kt{b}")
                vv = resid.tile([P, KC, HPC, HD + 1], BF16, name=f"v{b}")
                nc.vector.memset(vv[:, :, :, HD : HD + 1], 1.0)
                qt_sb.append(qt)
                kt_sb.append(kt)
                v_sb.append(vv)

            # ---------- projections ----------
            def qk_proj(b, r, which, pool_, tag):
                if which == "q":
                    w_t, b_t, dst = wq_sb, bq_sb, qt_sb[b]
                else:
                    w_t, b_t, dst = wk_sb, bk_sb, kt_sb[b]
                pp = pool_.tile([P, NFREE], F32, tag=tag, name=f"pp{which}{b}_{r}")
                for c in range(EC):
                    nc.tensor.matmul(
                        pp,
                        lhsT=w_t[:, c],
                        rhs=sl(which, b, r, c),
                        start=(c == 0),
                        stop=(not with_bias and c == EC - 1),
                    )
                if with_bias:
                    nc.tensor.matmul(pp, lhsT=b_t, rhs=ones_row, start=False, stop=True)
                nc.vector.tensor_copy(dst[:, r * NFREE : (r + 1) * NFREE], pp)

            def v_proj(b, r, half):
                for sub in (2 * half, 2 * half + 1):
                    kc = r * TPB + sub
                    pv = psmall.tile([P, DPC], F32, tag="ps", name=f"pv{b}_{kc}")
                    for c in range(EC):
                        nc.tensor.matmul(
                            pv,
                            lhsT=sl("e", b, r, c)[:, sub * P : (sub + 1) * P],
                            rhs=wv_sb[:, c],
                            start=(c == 0),
                            stop=(not with_bias and c == EC - 1),
                        )
                    if with_bias:
                        nc.tensor.matmul(
                            pv, lhsT=ones_row[:, :P], rhs=bv_sb, start=False, stop=True
                        )
                    for h in range(HPC):
                        nc.vector.tensor_copy(
                            v_sb[b][:, kc, h, 0:HD], pv[:, h * HD : (h + 1) * HD]
                        )

            # ---------- per-block ctx/norm/out tasks ----------
            pr_holder = {}
            T = _Task

            def make_ctx_tasks(bi):
                b, qc = ORDER[bi]
                base = NG * bi
                ctxps = {}
                ctxTs = {}
                ot = otp.tile([P, TPB, DPC], F32, tag="ot", name=f"ot{bi}")

                def ctx_chunk(h, j):
                    def run():
                        pr = pr_holder[bi]
                        if j == 0:
                            ctxps[h] = pctx.tile(
                                [HD + 1, NFREE], F32, tag="ctx", name=f"ctx{bi}_{h}"
                            )
                        cp = ctxps[h]
                        for kc in (2 * j, 2 * j + 1):
                            nc.tensor.matmul(
                                cp,
                                lhsT=v_sb[b][:, kc, h, :],
                                rhs=pr[:, 2 * kc + h, :],
                                start=(kc == 0),
                                stop=(kc == KC - 1),
                            )
                    return run

                def drain(h):
                    def run():
                        ctxTs[h] = misc.tile(
                            [HD + 1, NFREE], BF16, tag="ctxT", name=f"ctxT{bi}_{h}"
                        )
                        nc.vector.tensor_copy(ctxTs[h], ctxps[h])
                    return run

                def norm(h, tlo=0, thi=TPB):
                    def run():
                        nt = thi - tlo
                        tp = psmall.tile(
                            [P, nt, 80], BF16, tag="tp", name=f"tp{bi}_{h}_{tlo}"
                        )
                        for t in range(tlo, thi):
                            nc.tensor.transpose(
                                tp[:, t - tlo, 0 : HD + 1],
                                ctxTs[h][:, t * P : (t + 1) * P],
                                ident_bf[: HD + 1, : HD + 1],
                            )
                        rcp = misc.tile(
                            [P, nt, 1], F32, tag="rcp", bufs=4, name=f"rcp{bi}_{h}_{tlo}"
                        )
                        nc.vector.reciprocal(rcp, tp[:, :, HD : HD + 1])
                        nc.vector.tensor_mul(
                            ot[:, tlo:thi, h * HD : (h + 1) * HD],
                            tp[:, :, 0:HD],
                            rcp.broadcast_to([P, nt, HD]),
                        )
                    return run

                def dma_out(tlo, thi):
                    def run():
                        row0 = b * S + qc * NFREE + tlo * P
                        nc.sync.dma_start(
                            out.ap()[row0 : row0 + (thi - tlo) * P, :].rearrange(
                                "(t p) d -> p t d", p=P
                            ),
                            ot[:, tlo:thi, :],
                        )
                    return run

                tasks = []
                for j in range(8):
                    for h in range(HPC):
                        tasks.append(
                            T(base + 2 * j + 2 + h, base + 2 * j + 6, 430, ctx_chunk(h, j))
                        )
                if bi < NB - 1:
                    for h in range(HPC):
                        tasks.append(T(base + 17 + h, base + 22 + h, 100, drain(h)))
                        tasks.append(T(base + 18 + h, base + 26 + h, 600, norm(h)))
                    tasks.append(T(base + 20, base + 28, 0, dma_out(0, TPB)))
                else:
                    # fine-grained tail: 2-t norm chunks and split out-DMAs
                    # so the post-exp critical chain is as short as possible
                    tasks.append(T(base + 17, base + 23, 100, drain(0)))
                    tasks.append(T(base + 18, base + 24, 300, norm(0, 0, 2)))
                    tasks.append(T(base + 18, base + 25, 100, drain(1)))
                    tasks.append(T(base + 19, base + 25, 300, norm(0, 2, TPB)))
                    tasks.append(T(base + 19, base + 26, 300, norm(1, 0, 2)))
                    tasks.append(T(base + 20, base + 26, 0, dma_out(0, 2)))
                    tasks.append(T(base + 20, base + 27, 300, norm(1, 2, TPB)))
                    tasks.append(T(base + 21, base + 27, 0, dma_out(2, TPB)))
                return tasks

            # ---------- global filler task list ----------
            fillers = []

            def qk_task(b, r, which, gate, dl):
                fillers.append(
                    T(gate, dl, 1800, lambda b=b, r=r, w=which: qk_proj(b, r, w, psmall, "ps"))
                )

            def v_task(b, r, half, gate, dl):
                fillers.append(
                    T(gate, dl, 1040, lambda b=b, r=r, hf=half: v_proj(b, r, hf))
                )

            # b0 prep: e r1-3 proj feed score kc 4..15 (group == kc);
            # vproj(0,r,half) feeds ctx chunks (kc pair 2r+half, gate +2j+2)
            qk_task(0, 1, "e", 0, 4)
            qk_task(0, 2, "e", 1, 8)
            qk_task(0, 3, "e", 2, 12)
            for r in range(RC_B):
                for half in (0, 1):
                    j = 2 * r + half
                    v_task(0, r, half, j, 2 * j + 2)
            qk_task(0, 1, "q", 6, 16)
            qk_task(0, 2, "q", 18, 32)
            qk_task(0, 3, "q", 30, 48)

            # b1 source DMA issues (esrc/qsrc rings are deep enough that
            # slot reuse only needs b0-r-readers of the same slot emitted)
            def dma_b1(which, r, eng):
                def run():
                    dma_src(which, 1, r, 0, 4, eng)
                    dma_src(which, 1, r, 4, 8, eng)
                return run

            for r in range(RC_B):
                fillers.append(T(12 + r, 30 + 4 * r, 0, dma_b1("e", r, nc.sync)))
            # q-ring slot reuse: q1r2/q1r3 land on q02/q03 slots, whose
            # readers qproj(0,2)/(0,3) are emitted by G30/G46 (deadlines).
            for r, gate in enumerate((17, 18, 31, 47)):
                fillers.append(T(gate, 48 + 4 * r, 0, dma_b1("q", r, nc.gpsimd)))

            # b1 prep — gated as early as the b1 DMAs plausibly land, with
            # deadlines a few groups before the scores that need them so the
            # work spreads over blocks 2-3 instead of bunching at G62-79
            qk_task(1, 0, "e", 28, 54)
            qk_task(1, 1, "e", 32, 56)
            qk_task(1, 2, "e", 36, 58)
            qk_task(1, 3, "e", 40, 60)
            qk_task(1, 0, "q", 42, 60)
            for r in range(RC_B):
                for half in (0, 1):
                    j = 2 * r + half
                    v_task(1, r, half, 32 + 3 * j, 66 + 2 * j)
            qk_task(1, 1, "q", 56, 78)
            qk_task(1, 2, "q", 72, 94)
            qk_task(1, 3, "q", 88, 110)

            for bi in range(NB):
                fillers += make_ctx_tasks(bi)

            fillers.sort(key=lambda t: (t.gate, t.deadline))
            total_cost = sum(t.cost for t in fillers)

            # ---------- startup projections (r0 of b0) ----------
            qk_proj(0, 0, "e", spp, "sp")
            qk_proj(0, 0, "q", spp, "sp")

            # ---------- flat attention stream ----------
            def scores_for(X):
                bi, g = divmod(X, NG)
                b, qc = ORDER[bi]
                col0 = qc * NFREE
                sp = spp.tile([P, HPC, NFREE], F32, tag="sp", name=f"sp{X}")
                for h in range(HPC):
                    d0 = h * HD
                    nc.tensor.matmul(
                        sp[:, h, :],
                        lhsT=kt_sb[b][d0 : d0 + HD, g * P : (g + 1) * P],
                        rhs=qt_sb[b][d0 : d0 + HD, col0 : col0 + NFREE],
                        start=True,
                        stop=True,
                    )
                return sp

            sps = {0: scores_for(0), 1: scores_for(1)}
            done_cost = 0.0
            for G in range(NGT):
                bi, g = divmod(G, NG)
                if g == 0:
                    pr_holder[bi] = prp.tile(
                        [P, NSLOT, NFREE], BF16, tag="pr", name=f"pr{bi}"
                    )
                # forced pops: anything that must precede scores(G+2)
                i = 0
                while i < len(fillers):
                    if fillers[i].deadline <= G + 2:
                        t = fillers.pop(i)
                        t.fn()
                        done_cost += t.cost
                    else:
                        i += 1
                # budget pops: keep PE dense but never starve the exp stream
                # (front-loaded ~12% so prep work lands ahead of deadlines)
                want = total_cost * min(1.0, (G + 1) / (NGT * 0.88))
                while fillers and fillers[0].gate <= G and done_cost < want:
                    t = fillers.pop(0)
                    t.fn()
                    done_cost += t.cost
                if G + 2 < NGT:
                    sps[G + 2] = scores_for(G + 2)
                pr = pr_holder[bi]
                nc.scalar.activation(
                    pr[:, HPC * g : HPC * g + HPC, :], sps.pop(G), AF.Exp
                )
            while fillers:
                fillers.pop(0).fn()

    nc.finalize()
    return nc


def _get_nc(with_bias: bool = True) -> bass.Bass:
    if with_bias not in _CACHED_NC:
        _CACHED_NC[with_bias] = _build_nc(with_bias)
    return _CACHED_NC[with_bias]


def kernel(embed, q, Wk, bk, Wq, bq, Wv, bv, trace=False):
    global LAST_RESULTS
    bf = ml_dtypes.bfloat16
    embed = np.asarray(embed, dtype=np.float32)
    q = np.asarray(q, dtype=np.float32)
    Wk = np.asarray(Wk, dtype=np.float32)
    Wq = np.asarray(Wq, dtype=np.float32)
    Wv = np.asarray(Wv, dtype=np.float32)
    bk = np.asarray(bk, dtype=np.float32)
    bq = np.asarray(bq, dtype=np.float32)
    bv = np.asarray(bv, dtype=np.float32)

    qTt = _tile_inputs(q.reshape(ROWS, E))
    eTt = _tile_inputs(embed.reshape(ROWS, E))

    in_maps = []
    for c in range(NCORES):
        sl = slice(c * DPC, (c + 1) * DPC)
        in_maps.append(
            {
                "qTt": qTt,
                "eTt": eTt,
                # scores scale folded into Wq/bq (exact: *2^-3)
                "WqT": _tile_w(np.ascontiguousarray((Wq[sl] * SCALE).T).astype(bf)),
                "WkT": _tile_w(np.ascontiguousarray(Wk[sl].T).astype(bf)),
                "WvT": _tile_w(np.ascontiguousarray(Wv[sl].T).astype(bf)),
                "bqs": (bq[sl] * SCALE).astype(bf),
                "bkp": bk[sl].astype(bf),
                "bvp": bv[sl].astype(bf),
            }
        )

    with_bias = bool(bq.any() or bk.any() or bv.any())
    nc = _get_nc(with_bias)
    res = run_bass_kernel_spmd(nc, in_maps, list(range(NCORES)), trace=trace)
    LAST_RESULTS = res

    full = np.empty((ROWS, E), dtype=np.float32)
    for c in range(NCORES):
        full[:, c * DPC : (c + 1) * DPC] = res.results[c]["out"]
    return full.reshape(B, S, E)


# revision 25
# speedup vs baseline: 1.0207x; 1.0207x over previous
"""Cross multi-head attention TRN2 kernel (8-core SPMD, head-sharded), v2.

Strategy (tensor parallel over heads, zero communication):
  - 16 heads / 8 cores -> 2 heads per core. Core c computes output columns
    [128*c, 128*(c+1)) of the [4096, 1024] output; host concatenates.
  - Host pre-transposes and PRE-TILES q/embed into [tile, P, chunks] bf16
    layouts so every input DMA is contiguous per partition.
  - Scores are computed transposed (S^T[k, q] = K.Q^T, scale folded into Wq).
    The two heads per core live on partition halves 0-63 / 64-127, so their
    K=64 score matmuls row-tile and overlap ~2x on the PE.
  - The exp stream is 128 uniform groups of 2 score slots (one kc, both
    heads) x [P, 2, 512] fp32 PSUM, double buffered (4 banks); ACT is the
    near-critical engine (~130us) and the PE (~140us) paces the kernel.
  - All other PE work (attn@V 2-kc chunks, projections, V-proj, ctx
    transposes) is a task list with (gate, deadline) bounds popped between
    exp groups; deadlines are tight so work never bursts at block edges.
  - Softmax denominator via a ones-column appended to V (attn@V also
    produces row-sums); ctx'^T is PE-transposed back to [q, d] (bf16),
    normalized per-partition (DVE reciprocal+mul), DMA'd out per block.
"""

import numpy as np
import ml_dtypes

import concourse.bass as bass
import concourse.bacc as bacc
import concourse.mybir as mybir
import concourse.tile as tile
from concourse.bass_utils import run_bass_kernel_spmd
from concourse.masks import make_identity

# ---- problem dims (hardcoded; kernel.py must be self-contained) ----
B, S, E = 2, 2048, 1024
NHEAD, HD = 16, 64
NCORES = 8
HPC = NHEAD // NCORES          # heads per core = 2
DPC = HPC * HD                 # projection out-dims per core = 128
ROWS = B * S                   # 4096
P = 128                        # SBUF partitions
NFREE = 512                    # matmul moving free dim (one PSUM bank fp32)
EC = E // P                    # 8 contraction chunks
KC = S // P                    # 16 key chunks per batch
QC = S // NFREE                # 4 query chunks per batch
RC_B = S // NFREE              # 4 projection row-chunks per batch
TPB = NFREE // P               # 4 transpose chunks per block
NSLOT = HPC * KC               # 32 score slots per (b,qc) block
NG = KC                        # 16 exp groups per block (2 slots each)
NB = B * QC                    # 8 blocks
NGT = NB * NG                  # 128 groups total
SCALE = 1.0 / np.sqrt(HD)      # 0.125, folded into Wq/bq on host

F32 = mybir.dt.float32
BF16 = mybir.dt.bfloat16
AF = mybir.ActivationFunctionType

_CACHED_NC = {}
LAST_RESULTS = None            # test.py reads exec_time_ns / profile from here

ORDER = [(0, 0), (0, 1), (0, 2), (0, 3), (1, 0), (1, 1), (1, 2), (1, 3)]


def _tile_inputs(mat_rows_e: np.ndarray) -> np.ndarray:
    """[ROWS, E] f32 -> pre-tiled bf16 [P, B*RC_B, EC*NFREE]: slice
    [:, b*RC_B+r, :] is one projection row-chunk, 8KB contiguous per
    partition, laid out [c, n] with E-index = c*128 + p, col = b*S +
    r*512 + n."""
    t = np.ascontiguousarray(mat_rows_e.T).astype(ml_dtypes.bfloat16)
    a = t.reshape(EC, P, B, RC_B, NFREE)            # [c, p, b, r, n]
    a = a.transpose(1, 2, 3, 0, 4)                  # [p, b, r, c, n]
    return np.ascontiguousarray(a.reshape(P, B * RC_B, EC * NFREE))


def _tile_w(wT: np.ndarray) -> np.ndarray:
    """[E, DPC] bf16 weight -> [P, EC*DPC]: contiguous per partition."""
    a = wT.reshape(EC, P, DPC).transpose(1, 0, 2)
    return np.ascontiguousarray(a.reshape(P, EC * DPC))


class _Task:
    """Filler work item: gate = earliest group index after whose exp it may
    be emitted; deadline = group whose SCORES it must precede (forced-pop
    at G >= deadline-2); cost = PE-ns estimate for pacing."""

    __slots__ = ("gate", "deadline", "cost", "fn")

    def __init__(self, gate, deadline, cost, fn):
        self.gate = gate
        self.deadline = deadline
        self.cost = cost
        self.fn = fn


def _build_nc(with_bias: bool) -> bass.Bass:
    nc = bacc.Bacc(
        "TRN2",
        target_bir_lowering=False,
        debug=False,
        num_devices=NCORES,
    )

    qTt = nc.declare_dram_parameter("qTt", [P, B * RC_B, EC * NFREE], BF16, isOutput=False)
    eTt = nc.declare_dram_parameter("eTt", [P, B * RC_B, EC * NFREE], BF16, isOutput=False)
    WqT = nc.declare_dram_parameter("WqT", [P, EC * DPC], BF16, isOutput=False)
    WkT = nc.declare_dram_parameter("WkT", [P, EC * DPC], BF16, isOutput=False)
    WvT = nc.declare_dram_parameter("WvT", [P, EC * DPC], BF16, isOutput=False)
    bqs = nc.declare_dram_parameter("bqs", [DPC], BF16, isOutput=False)
    bkp = nc.declare_dram_parameter("bkp", [DPC], BF16, isOutput=False)
    bvp = nc.declare_dram_parameter("bvp", [DPC], BF16, isOutput=False)
    out = nc.declare_dram_parameter("out", [ROWS, DPC], F32, isOutput=True)

    with tile.TileContext(nc) as tc:
        with (
            tc.tile_pool(name="consts", bufs=1) as consts,
            tc.tile_pool(name="wpool", bufs=1) as wpool,
            tc.tile_pool(name="resid", bufs=1) as resid,
            tc.tile_pool(name="esrc", bufs=1) as esrc,
            tc.tile_pool(name="qsrc", bufs=1) as qsrc,
            tc.tile_pool(name="prp", bufs=2) as prp,
            tc.tile_pool(name="misc", bufs=2) as misc,
            tc.tile_pool(name="otp", bufs=4) as otp,
            # PSUM banks: sp 2x2 + ctx 2 + ps 1 + tp 1 = 8
            tc.tile_pool(name="spp", bufs=2, space="PSUM") as spp,
            tc.tile_pool(name="pctx", bufs=2, space="PSUM") as pctx,
            tc.tile_pool(name="psmall", bufs=1, space="PSUM") as psmall,
        ):
            # ---------- weights first (first c-chunks split out so the
            # very first projection matmuls can start early) ----
            wk_sb = wpool.tile([P, EC, DPC], BF16, name="wk_sb")
            nc.sync.dma_start(
                wk_sb[:, 0:2], WkT.ap()[:, : 2 * DPC].rearrange("p (c d) -> p c d", c=2)
            )
            wq_sb = wpool.tile([P, EC, DPC], BF16, name="wq_sb")
            nc.scalar.dma_start(
                wq_sb[:, 0:2], WqT.ap()[:, : 2 * DPC].rearrange("p (c d) -> p c d", c=2)
            )
            nc.sync.dma_start(
                wk_sb[:, 2:], WkT.ap()[:, 2 * DPC :].rearrange("p (c d) -> p c d", c=EC - 2)
            )
            nc.scalar.dma_start(
                wq_sb[:, 2:], WqT.ap()[:, 2 * DPC :].rearrange("p (c d) -> p c d", c=EC - 2)
            )

            # ---------- source DMAs (chunked along E-contraction) --------
            src_chunks = {}

            def dma_src(which, b, r, lo, hi, eng):
                dram = qTt if which == "q" else eTt
                tag = f"{which}{hi - lo}"
                tl = (qsrc if which == "q" else esrc).tile(
                    [P, hi - lo, NFREE], BF16, tag=tag, bufs=(4 if hi - lo == 2 else 8),
                    name=f"{which}{b}{r}c{lo}",
                )
                eng.dma_start(
                    tl,
                    dram.ap()[:, b * RC_B + r, lo * NFREE : hi * NFREE].rearrange(
                        "p (c n) -> p c n", c=hi - lo
                    ),
                )
                src_chunks.setdefault((which, b, r), []).append((tl, lo, hi))

            def sl(which, b, r, c):
                for tl, lo, hi in src_chunks[(which, b, r)]:
                    if lo <= c < hi:
                        return tl[:, c - lo]
                raise KeyError((which, b, r, c))

            # startup-critical quarters: e00 on sync+scalar, q00 on
            # gpsimd+scalar (3 DMA-capable rings: sync, scalar, gpsimd)
            dma_src("e", 0, 0, 0, 2, nc.sync)
            dma_src("e", 0, 0, 4, 6, nc.scalar)
            dma_src("q", 0, 0, 0, 2, nc.gpsimd)
            dma_src("e", 0, 0, 2, 4, nc.sync)
            dma_src("e", 0, 0, 6, 8, nc.scalar)
            dma_src("q", 0, 0, 2, 4, nc.gpsimd)
            dma_src("q", 0, 0, 4, 6, nc.sync)
            dma_src("q", 0, 0, 6, 8, nc.sync)
            # wv + biases on gpsimd (needed by vproj from G0ish — must be
            # queued ahead of the bulky q01-03 halves)
            wv_sb = wpool.tile([P, EC, DPC], BF16, name="wv_sb")
            nc.gpsimd.dma_start(wv_sb, WvT.ap().rearrange("p (c d) -> p c d", c=EC))
            bq_sb = wpool.tile([1, DPC], BF16, name="bq_sb")
            nc.gpsimd.dma_start(bq_sb, bqs.ap()[None, :])
            bk_sb = wpool.tile([1, DPC], BF16, name="bk_sb")
            nc.gpsimd.dma_start(bk_sb, bkp.ap()[None, :])
            bv_sb = wpool.tile([1, DPC], BF16, name="bv_sb")
            nc.gpsimd.dma_start(bv_sb, bvp.ap()[None, :])
            # rest of b0 as halves; the first (c0-3) half of each e-tile
            # goes on the fast scalar ring so kproj can start per-tile early
            for r in (1, 2, 3):
                dma_src("e", 0, r, 0, 4, nc.scalar)
                dma_src("e", 0, r, 4, 8, nc.sync)
            for r in (1, 2, 3):
                dma_src("q", 0, r, 0, 4, nc.gpsimd)
                dma_src("q", 0, r, 4, 8, nc.gpsimd)

            # ---------- constants ----------
            ones_row = consts.tile([1, NFREE], BF16)
            nc.vector.memset(ones_row, 1.0)
            # warm the PE HAM (flip to 2.4 GHz) during the initial DMA wait.
            # High-duty N=512 matmuls (the HAM watches array streaming; tiny
            # N keeps it cold) spanning ~4us, ending about when the first
            # input chunks land so the real projections run at full clock.
            wmp = psmall.tile([P, NFREE], F32, tag="ps", name="wmp")
            for _ in range(11):
                nc.tensor.matmul(
                    wmp, lhsT=ones_row[:, 0:P], rhs=ones_row,
                    start=True, stop=True,
                )
            ident_bf = consts.tile([P, P], BF16)
            make_identity(nc, ident_bf)
            # warm the ACT exp table (after scalar's DMA issues)
            warm = consts.tile([1, 1], BF16)
            nc.scalar.activation(warm, ones_row[:, 0:1], AF.Exp)

            # ---------- residents (per batch) ----------
            qt_sb = []
            kt_sb = []
            v_sb = []
            for b in range(B):
                qt = resid.tile([P, S], BF16, name=f"qt{b}")
                kt = resid.tile([P, S], BF16, name=f"kt{b}")
                vv = resid.tile([P, KC, HPC, HD + 1], BF16, name=f"v{b}")
                nc.vector.memset(vv[:, :, :, HD : HD + 1], 1.0)
                qt_sb.append(qt)
                kt_sb.append(kt)
                v_sb.append(vv)

            # ---------- projections ----------
            def qk_proj(b, r, which, pool_, tag):
                if which == "q":
                    w_t, b_t, dst = wq_sb, bq_sb, qt_sb[b]
                else:
                    w_t, b_t, dst = wk_sb, bk_sb, kt_sb[b]
                pp = pool_.tile([P, NFREE], F32, tag=tag, name=f"pp{which}{b}_{r}")
                for c in range(EC):
                    nc.tensor.matmul(
                        pp,
                        lhsT=w_t[:, c],
                        rhs=sl(which, b, r, c),
                        start=(c == 0),
                        stop=(not with_bias and c == EC - 1),
                    )
                if with_bias:
                    nc.tensor.matmul(pp, lhsT=b_t, rhs=ones_row, start=False, stop=True)
                nc.vector.tensor_copy(dst[:, r * NFREE : (r + 1) * NFREE], pp)

            def v_proj(b, r, half):
                for sub in (2 * half, 2 * half + 1):
                    kc = r * TPB + sub
                    pv = psmall.tile([P, DPC], F32, tag="ps", name=f"pv{b}_{kc}")
                    for c in range(EC):
                        nc.tensor.matmul(
                            pv,
                            lhsT=sl("e", b, r, c)[:, sub * P : (sub + 1) * P],
                            rhs=wv_sb[:, c],
                            start=(c == 0),
                            stop=(not with_bias and c == EC - 1),
                        )
                    if with_bias:
                        nc.tensor.matmul(
                            pv, lhsT=ones_row[:, :P], rhs=bv_sb, start=False, stop=True
                        )
                    for h in range(HPC):
                        nc.vector.tensor_copy(
                            v_sb[b][:, kc, h, 0:HD], pv[:, h * HD : (h + 1) * HD]
                        )

            # ---------- per-block ctx/norm/out tasks ----------
            pr_holder = {}
            T = _Task

            def make_ctx_tasks(bi):
                b, qc = ORDER[bi]
                base = NG * bi
                ctxps = {}
                ctxTs = {}
                ot = otp.tile([P, TPB, DPC], F32, tag="ot", name=f"ot{bi}")

                def ctx_chunk(h, j):
                    def run():
                        pr = pr_holder[bi]
                        if j == 0:
                            ctxps[h] = pctx.tile(
                                [HD + 1, NFREE], F32, tag="ctx", name=f"ctx{bi}_{h}"
                            )
                        cp = ctxps[h]
                        for kc in (2 * j, 2 * j + 1):
                            nc.tensor.matmul(
                                cp,
                                lhsT=v_sb[b][:, kc, h, :],
                                rhs=pr[:, 2 * kc + h, :],
                                start=(kc == 0),
                                stop=(kc == KC - 1),
                            )
                    return run

                def drain(h):
                    def run():
                        ctxTs[h] = misc.tile(
                            [HD + 1, NFREE], BF16, tag="ctxT", name=f"ctxT{bi}_{h}"
                        )
                        nc.vector.tensor_copy(ctxTs[h], ctxps[h])
                    return run

                def norm(h, tlo=0, thi=TPB, ptag="tp"):
                    def run():
                        nt = thi - tlo
                        tp = psmall.tile(
                            [P, nt, 80], BF16, tag=ptag, name=f"tp{bi}_{h}_{tlo}"
                        )
                        for t in range(tlo, thi):
                            nc.tensor.transpose(
                                tp[:, t - tlo, 0 : HD + 1],
                                ctxTs[h][:, t * P : (t + 1) * P],
                                ident_bf[: HD + 1, : HD + 1],
                            )
                        rcp = misc.tile(
                            [P, nt, 1], F32, tag="rcp", bufs=4, name=f"rcp{bi}_{h}_{tlo}"
                        )
                        nc.vector.reciprocal(rcp, tp[:, :, HD : HD + 1])
                        nc.vector.tensor_mul(
                            ot[:, tlo:thi, h * HD : (h + 1) * HD],
                            tp[:, :, 0:HD],
                            rcp.broadcast_to([P, nt, HD]),
                        )
                    return run

                def dma_out(tlo, thi):
                    def run():
                        row0 = b * S + qc * NFREE + tlo * P
                        nc.sync.dma_start(
                            out.ap()[row0 : row0 + (thi - tlo) * P, :].rearrange(
                                "(t p) d -> p t d", p=P
                            ),
                            ot[:, tlo:thi, :],
                        )
                    return run

                tasks = []
                # b0's e-tiles (-> V kc8-15) only land ~G12-15, so block 0's
                # late ctx chunks shift into block 1's window; block 1 then
                # waits for block 0's pctx drains, block 2 half-recovers and
                # block 3+ runs the steady-state template.
                if bi == 0:
                    cg = [4, 6, 8, 10, 20, 22, 24, 26]
                    dr, nr, dm = 28, 30, 32
                elif bi == 1:
                    cg = [30, 31, 32, 33, 34, 35, 36, 37]
                    dr, nr, dm = 38, 40, 42
                elif bi == 2:
                    # j7 reads group 47's slots -> gate 48 (after exp(47))
                    cg = [40, 41, 42, 43, 44, 45, 46, 48]
                    dr, nr, dm = 49, 50, 52
                else:
                    cg = [base + 2 * j + 2 for j in range(8)]
                    dr, nr, dm = base + 17, base + 18, base + 20
                for j in range(8):
                    for h in range(HPC):
                        tasks.append(T(cg[j] + h, cg[j] + h + 4, 430, ctx_chunk(h, j)))
                if bi < NB - 1:
                    for h in range(HPC):
                        tasks.append(T(dr + h, dr + h + 4, 100, drain(h)))
                        tasks.append(T(nr + h, nr + h + 5, 600, norm(h)))
                    tasks.append(T(dm, dm + 6, 0, dma_out(0, TPB)))
                else:
                    # fine-grained tail: 2-t norm chunks and split out-DMAs
                    # so the post-exp critical chain is as short as possible
                    tasks.append(T(base + 17, base + 23, 100, drain(0)))
                    tasks.append(T(base + 18, base + 24, 300, norm(0, 0, 2, "tp")))
                    tasks.append(T(base + 18, base + 25, 100, drain(1)))
                    tasks.append(T(base + 19, base + 25, 300, norm(0, 2, TPB, "ps")))
                    tasks.append(T(base + 19, base + 26, 300, norm(1, 0, 2, "tp")))
                    tasks.append(T(base + 20, base + 26, 0, dma_out(0, 2)))
                    tasks.append(T(base + 20, base + 27, 300, norm(1, 2, TPB, "ps")))
                    tasks.append(T(base + 21, base + 27, 0, dma_out(2, TPB)))
                return tasks

            # ---------- global filler task list ----------
            fillers = []

            def qk_task(b, r, which, gate, dl):
                fillers.append(
                    T(gate, dl, 1800, lambda b=b, r=r, w=which: qk_proj(b, r, w, psmall, "ps"))
                )

            def v_task(b, r, half, gate, dl):
                fillers.append(
                    T(gate, dl, 1040, lambda b=b, r=r, hf=half: v_proj(b, r, hf))
                )

            # b0 prep: e r1-3 proj feed score kc 4..15 (group == kc); gates
            # match expected DMA arrival so pops don't block the PE queue
            qk_task(0, 1, "e", 0, 4)
            qk_task(0, 2, "e", 4, 8)
            qk_task(0, 3, "e", 8, 12)
            v_task(0, 0, 0, 0, 4)
            v_task(0, 0, 1, 1, 6)
            v_task(0, 1, 0, 4, 8)
            v_task(0, 1, 1, 5, 10)
            v_task(0, 2, 0, 8, 20)
            v_task(0, 2, 1, 9, 22)
            v_task(0, 3, 0, 12, 24)
            v_task(0, 3, 1, 13, 26)
            qk_task(0, 1, "q", 6, 16)
            qk_task(0, 2, "q", 18, 32)
            qk_task(0, 3, "q", 30, 48)

            # b1 source DMA issues (esrc/qsrc rings are deep enough that
            # slot reuse only needs b0-r-readers of the same slot emitted)
            def dma_b1(which, r, eng):
                def run():
                    dma_src(which, 1, r, 0, 4, eng)
                    dma_src(which, 1, r, 4, 8, eng)
                return run

            for r in range(RC_B):
                fillers.append(T(12 + r, 30 + 4 * r, 0, dma_b1("e", r, nc.sync)))
            # q-ring slot reuse: q1r2/q1r3 land on q02/q03 slots, whose
            # readers qproj(0,2)/(0,3) are emitted by G30/G46 (deadlines).
            for r, gate in enumerate((17, 18, 31, 47)):
                fillers.append(T(gate, 48 + 4 * r, 0, dma_b1("q", r, nc.gpsimd)))

            # b1 prep — gated as early as the b1 DMAs plausibly land, with
            # deadlines a few groups before the scores that need them so the
            # work spreads over blocks 2-3 instead of bunching at G62-79
            qk_task(1, 0, "e", 28, 54)
            qk_task(1, 1, "e", 32, 56)
            qk_task(1, 2, "e", 36, 58)
            qk_task(1, 3, "e", 40, 60)
            qk_task(1, 0, "q", 42, 60)
            # vproj(1) is only needed from G66 — park it in the light back
            # half of the schedule instead of the oversubscribed front
            for r in range(RC_B):
                for half in (0, 1):
                    j = 2 * r + half
                    v_task(1, r, half, 50 + 2 * j, 66 + 2 * j)
            qk_task(1, 1, "q", 64, 78)
            qk_task(1, 2, "q", 72, 94)
            qk_task(1, 3, "q", 88, 110)

            for bi in range(NB):
                fillers += make_ctx_tasks(bi)

            fillers.sort(key=lambda t: (t.gate, t.deadline))

            # ---------- startup projections (r0 of b0) ----------
            qk_proj(0, 0, "e", spp, "sp")
            qk_proj(0, 0, "q", spp, "sp")

            # ---------- flat attention stream ----------
            def scores_for(X):
                bi, g = divmod(X, NG)
                b, qc = ORDER[bi]
                col0 = qc * NFREE
                sp = spp.tile([P, HPC, NFREE], F32, tag="sp", name=f"sp{X}")
                for h in range(HPC):
                    d0 = h * HD
                    nc.tensor.matmul(
                        sp[:, h, :],
                        lhsT=kt_sb[b][d0 : d0 + HD, g * P : (g + 1) * P],
                        rhs=qt_sb[b][d0 : d0 + HD, col0 : col0 + NFREE],
                        start=True,
                        stop=True,
                    )
                return sp

            sps = {0: scores_for(0), 1: scores_for(1)}
            for G in range(NGT):
                bi, g = divmod(G, NG)
                if g == 0:
                    pr_holder[bi] = prp.tile(
                        [P, NSLOT, NFREE], BF16, tag="pr", name=f"pr{bi}"
                    )
                # forced pops: anything that must precede scores(G+2)
                i = 0
                while i < len(fillers):
                    if fillers[i].deadline <= G + 2:
                        fillers.pop(i).fn()
                    else:
                        i += 1
                # gate pops: gates encode both data arrival and load spread,
                # so pop everything that is ready
                while fillers and fillers[0].gate <= G:
                    fillers.pop(0).fn()
                if G + 2 < NGT:
                    sps[G + 2] = scores_for(G + 2)
                pr = pr_holder[bi]
                nc.scalar.activation(
                    pr[:, HPC * g : HPC * g + HPC, :], sps.pop(G), AF.Exp
                )
            while fillers:
                fillers.pop(0).fn()

    nc.finalize()
    return nc


def _get_nc(with_bias: bool = True) -> bass.Bass:
    if with_bias not in _CACHED_NC:
        _CACHED_NC[with_bias] = _build_nc(with_bias)
    return _CACHED_NC[with_bias]


def kernel(embed, q, Wk, bk, Wq, bq, Wv, bv, trace=False):
    global LAST_RESULTS
    bf = ml_dtypes.bfloat16
    embed = np.asarray(embed, dtype=np.float32)
    q = np.asarray(q, dtype=np.float32)
    Wk = np.asarray(Wk, dtype=np.float32)
    Wq = np.asarray(Wq, dtype=np.float32)
    Wv = np.asarray(Wv, dtype=np.float32)
    bk = np.asarray(bk, dtype=np.float32)
    bq = np.asarray(bq, dtype=np.float32)
    bv = np.asarray(bv, dtype=np.float32)

    qTt = _tile_inputs(q.reshape(ROWS, E))
    eTt = _tile_inputs(embed.reshape(ROWS, E))

    in_maps = []
    for c in range(NCORES):
        sl = slice(c * DPC, (c + 1) * DPC)
        in_maps.append(
            {
                "qTt": qTt,
                "eTt": eTt,
                # scores scale folded into Wq/bq (exact: *2^-3)
                "WqT": _tile_w(np.ascontiguousarray((Wq[sl] * SCALE).T).astype(bf)),
                "WkT": _tile_w(np.ascontiguousarray(Wk[sl].T).astype(bf)),
                "WvT": _tile_w(np.ascontiguousarray(Wv[sl].T).astype(bf)),
                "bqs": (bq[sl] * SCALE).astype(bf),
                "bkp": bk[sl].astype(bf),
                "bvp": bv[sl].astype(bf),
            }
        )

    with_bias = bool(bq.any() or bk.any() or bv.any())
    nc = _get_nc(with_bias)
    res = run_bass_kernel_spmd(nc, in_maps, list(range(NCORES)), trace=trace)
    LAST_RESULTS = res

    full = np.empty((ROWS, E), dtype=np.float32)
    for c in range(NCORES):
        full[:, c * DPC : (c + 1) * DPC] = res.results[c]["out"]
    return full.reshape(B, S, E)


# revision 33
# speedup vs baseline: 1.0294x; 1.0085x over previous
"""Cross multi-head attention TRN2 kernel (8-core SPMD, head-sharded), v2.

Strategy (tensor parallel over heads, zero communication):
  - 16 heads / 8 cores -> 2 heads per core. Core c computes output columns
    [128*c, 128*(c+1)) of the [4096, 1024] output; host concatenates.
  - Host pre-transposes and PRE-TILES q/embed into [tile, P, chunks] bf16
    layouts so every input DMA is contiguous per partition.
  - Scores are computed transposed (S^T[k, q] = K.Q^T, scale folded into Wq).
    The two heads per core live on partition halves 0-63 / 64-127, so their
    K=64 score matmuls row-tile and overlap ~2x on the PE.
  - The exp stream is 128 uniform groups of 2 score slots (one kc, both
    heads) x [P, 2, 512] fp32 PSUM, double buffered (4 banks); ACT is the
    near-critical engine (~130us) and the PE (~140us) paces the kernel.
  - All other PE work (attn@V 2-kc chunks, projections, V-proj, ctx
    transposes) is a task list with (gate, deadline) bounds popped between
    exp groups; deadlines are tight so work never bursts at block edges.
  - Softmax denominator via a ones-column appended to V (attn@V also
    produces row-sums); ctx'^T is PE-transposed back to [q, d] (bf16),
    normalized per-partition (DVE reciprocal+mul), DMA'd out per block.
"""

import numpy as np
import ml_dtypes

import concourse.bass as bass
import concourse.bacc as bacc
import concourse.mybir as mybir
import concourse.tile as tile
from concourse.bass_utils import run_bass_kernel_spmd

# ---- problem dims (hardcoded; kernel.py must be self-contained) ----
B, S, E = 2, 2048, 1024
NHEAD, HD = 16, 64
NCORES = 8
HPC = NHEAD // NCORES          # heads per core = 2
DPC = HPC * HD                 # projection out-dims per core = 128
ROWS = B * S                   # 4096
P = 128                        # SBUF partitions
NFREE = 512                    # matmul moving free dim (one PSUM bank fp32)
EC = E // P                    # 8 contraction chunks
KC = S // P                    # 16 key chunks per batch
QC = S // NFREE                # 4 query chunks per batch
RC_B = S // NFREE              # 4 projection row-chunks per batch
TPB = NFREE // P               # 4 transpose chunks per block
NSLOT = HPC * KC               # 32 score slots per (b,qc) block
NG = KC                        # 16 exp groups per block (2 slots each)
NB = B * QC                    # 8 blocks
NGT = NB * NG                  # 128 groups total
SCALE = 1.0 / np.sqrt(HD)      # 0.125, folded into Wq/bq on host

F32 = mybir.dt.float32
BF16 = mybir.dt.bfloat16
AF = mybir.ActivationFunctionType

_CACHED_NC = {}
LAST_RESULTS = None            # test.py reads exec_time_ns / profile from here

ORDER = [(0, 0), (0, 1), (0, 2), (0, 3), (1, 0), (1, 1), (1, 2), (1, 3)]


def _tile_inputs(mat_rows_e: np.ndarray) -> np.ndarray:
    """[ROWS, E] f32 -> pre-tiled bf16 [P, B*RC_B, EC*NFREE]: slice
    [:, b*RC_B+r, :] is one projection row-chunk, 8KB contiguous per
    partition, laid out [c, n] with E-index = c*128 + p, col = b*S +
    r*512 + n."""
    t = np.ascontiguousarray(mat_rows_e.T).astype(ml_dtypes.bfloat16)
    a = t.reshape(EC, P, B, RC_B, NFREE)            # [c, p, b, r, n]
    a = a.transpose(1, 2, 3, 0, 4)                  # [p, b, r, c, n]
    return np.ascontiguousarray(a.reshape(P, B * RC_B, EC * NFREE))


def _tile_w(wT: np.ndarray) -> np.ndarray:
    """[E, DPC] bf16 weight -> [P, EC*DPC]: contiguous per partition."""
    a = wT.reshape(EC, P, DPC).transpose(1, 0, 2)
    return np.ascontiguousarray(a.reshape(P, EC * DPC))


class _Task:
    """Filler work item: gate = earliest group index after whose exp it may
    be emitted; deadline = group whose SCORES it must precede (forced-pop
    at G >= deadline-2); cost = PE-ns estimate for pacing."""

    __slots__ = ("gate", "deadline", "cost", "fn")

    def __init__(self, gate, deadline, cost, fn):
        self.gate = gate
        self.deadline = deadline
        self.cost = cost
        self.fn = fn


def _build_nc(with_bias: bool) -> bass.Bass:
    nc = bacc.Bacc(
        "TRN2",
        target_bir_lowering=False,
        debug=False,
        num_devices=NCORES,
    )

    qTt = nc.declare_dram_parameter("qTt", [P, B * RC_B, EC * NFREE], BF16, isOutput=False)
    eTt = nc.declare_dram_parameter("eTt", [P, B * RC_B, EC * NFREE], BF16, isOutput=False)
    WqT = nc.declare_dram_parameter("WqT", [P, EC * DPC], BF16, isOutput=False)
    WkT = nc.declare_dram_parameter("WkT", [P, EC * DPC], BF16, isOutput=False)
    WvT = nc.declare_dram_parameter("WvT", [P, EC * DPC], BF16, isOutput=False)
    bqs = nc.declare_dram_parameter("bqs", [DPC], BF16, isOutput=False)
    bkp = nc.declare_dram_parameter("bkp", [DPC], BF16, isOutput=False)
    bvp = nc.declare_dram_parameter("bvp", [DPC], BF16, isOutput=False)
    idn = nc.declare_dram_parameter("idn", [P, P], BF16, isOutput=False)
    out = nc.declare_dram_parameter("out", [ROWS, DPC], F32, isOutput=True)

    with tile.TileContext(nc) as tc:
        with (
            tc.tile_pool(name="consts", bufs=1) as consts,
            tc.tile_pool(name="wpool", bufs=1) as wpool,
            tc.tile_pool(name="resid", bufs=1) as resid,
            tc.tile_pool(name="esrc", bufs=1) as esrc,
            tc.tile_pool(name="qsrc", bufs=1) as qsrc,
            tc.tile_pool(name="prp", bufs=2) as prp,
            tc.tile_pool(name="misc", bufs=2) as misc,
            tc.tile_pool(name="otp", bufs=4) as otp,
            # PSUM banks: sp 2x2 + ctx 2 + ps 1 + tp 1 = 8
            tc.tile_pool(name="spp", bufs=2, space="PSUM") as spp,
            tc.tile_pool(name="pctx", bufs=2, space="PSUM") as pctx,
            tc.tile_pool(name="psmall", bufs=1, space="PSUM") as psmall,
        ):
            # ---------- weights first (first c-chunks split out so the
            # very first projection matmuls can start early) ----
            wk_sb = wpool.tile([P, EC, DPC], BF16, name="wk_sb")
            nc.sync.dma_start(
                wk_sb[:, 0:2], WkT.ap()[:, : 2 * DPC].rearrange("p (c d) -> p c d", c=2)
            )
            # identity via DMA (host-prepared): ready ~9us, feeds the PE
            # warm-up chain — gpsimd make_identity would land far too late
            ident_bf = consts.tile([P, P], BF16, name="ident_bf")
            nc.scalar.dma_start(ident_bf, idn.ap())
            wq_sb = wpool.tile([P, EC, DPC], BF16, name="wq_sb")
            nc.scalar.dma_start(
                wq_sb[:, 0:2], WqT.ap()[:, : 2 * DPC].rearrange("p (c d) -> p c d", c=2)
            )
            nc.sync.dma_start(
                wk_sb[:, 2:], WkT.ap()[:, 2 * DPC :].rearrange("p (c d) -> p c d", c=EC - 2)
            )
            nc.scalar.dma_start(
                wq_sb[:, 2:], WqT.ap()[:, 2 * DPC :].rearrange("p (c d) -> p c d", c=EC - 2)
            )

            # ---------- source DMAs (chunked along E-contraction) --------
            src_chunks = {}

            def dma_src(which, b, r, lo, hi, eng):
                dram = qTt if which == "q" else eTt
                tag = f"{which}{hi - lo}"
                tl = (qsrc if which == "q" else esrc).tile(
                    [P, hi - lo, NFREE], BF16, tag=tag, bufs=(4 if hi - lo == 2 else 8),
                    name=f"{which}{b}{r}c{lo}",
                )
                eng.dma_start(
                    tl,
                    dram.ap()[:, b * RC_B + r, lo * NFREE : hi * NFREE].rearrange(
                        "p (c n) -> p c n", c=hi - lo
                    ),
                )
                src_chunks.setdefault((which, b, r), []).append((tl, lo, hi))

            def sl(which, b, r, c):
                for tl, lo, hi in src_chunks[(which, b, r)]:
                    if lo <= c < hi:
                        return tl[:, c - lo]
                raise KeyError((which, b, r, c))

            # startup-critical quarters: e00 on sync+scalar, q00 on
            # gpsimd+scalar (3 DMA-capable rings: sync, scalar, gpsimd)
            dma_src("e", 0, 0, 0, 2, nc.sync)
            dma_src("e", 0, 0, 4, 6, nc.scalar)
            dma_src("q", 0, 0, 0, 2, nc.gpsimd)
            dma_src("e", 0, 0, 2, 4, nc.sync)
            dma_src("e", 0, 0, 6, 8, nc.scalar)
            dma_src("q", 0, 0, 2, 4, nc.gpsimd)
            dma_src("q", 0, 0, 4, 6, nc.sync)
            dma_src("q", 0, 0, 6, 8, nc.sync)
            # wv + biases on gpsimd (needed by vproj from G0ish — must be
            # queued ahead of the bulky q01-03 halves)
            wv_sb = wpool.tile([P, EC, DPC], BF16, name="wv_sb")
            nc.gpsimd.dma_start(wv_sb, WvT.ap().rearrange("p (c d) -> p c d", c=EC))
            bq_sb = wpool.tile([1, DPC], BF16, name="bq_sb")
            nc.gpsimd.dma_start(bq_sb, bqs.ap()[None, :])
            bk_sb = wpool.tile([1, DPC], BF16, name="bk_sb")
            nc.gpsimd.dma_start(bk_sb, bkp.ap()[None, :])
            bv_sb = wpool.tile([1, DPC], BF16, name="bv_sb")
            nc.gpsimd.dma_start(bv_sb, bvp.ap()[None, :])
            # rest of b0 as halves; the first (c0-3) half of each e-tile
            # goes on the fast scalar ring so kproj can start per-tile early
            for r in (1, 2, 3):
                dma_src("e", 0, r, 0, 4, nc.scalar)
                dma_src("e", 0, r, 4, 8, nc.sync)
            for r in (1, 2, 3):
                dma_src("q", 0, r, 0, 4, nc.gpsimd)
                dma_src("q", 0, r, 4, 8, nc.gpsimd)

            # ---------- constants ----------
            ones_row = consts.tile([1, NFREE], BF16)
            nc.vector.memset(ones_row, 1.0)
            # warm the PE HAM (flip to 2.4 GHz) during the initial DMA wait.
            # K=128 dense matmuls on the identity (K=1 or tiny-N chains keep
            # the array duty too low to trip the HAM busy window).
            wmp = psmall.tile([P, NFREE], F32, tag="ps", name="wmp")

            def warm_mm(n):
                for _ in range(n):
                    nc.tensor.matmul(
                        wmp[:, 0:P], lhsT=ident_bf, rhs=ident_bf,
                        start=True, stop=True,
                    )

            warm_mm(16)
            # warm the ACT exp table (after scalar's DMA issues)
            warm = consts.tile([1, 1], BF16)
            nc.scalar.activation(warm, ones_row[:, 0:1], AF.Exp)

            # ---------- residents (per batch) ----------
            qt_sb = []
            kt_sb = []
            v_sb = []
            for b in range(B):
                qt = resid.tile([P, S], BF16, name=f"qt{b}")
                kt = resid.tile([P, S], BF16, name=f"kt{b}")
                vv = resid.tile([P, KC, HPC, HD + 1], BF16, name=f"v{b}")
                nc.vector.memset(vv[:, :, :, HD : HD + 1], 1.0)
                qt_sb.append(qt)
                kt_sb.append(kt)
                v_sb.append(vv)

            # ---------- projections ----------
            def qk_proj(b, r, which, pool_, tag):
                if which == "q":
                    w_t, b_t, dst = wq_sb, bq_sb, qt_sb[b]
                else:
                    w_t, b_t, dst = wk_sb, bk_sb, kt_sb[b]
                pp = pool_.tile([P, NFREE], F32, tag=tag, name=f"pp{which}{b}_{r}")
                for c in range(EC):
                    nc.tensor.matmul(
                        pp,
                        lhsT=w_t[:, c],
                        rhs=sl(which, b, r, c),
                        start=(c == 0),
                        stop=(not with_bias and c == EC - 1),
                    )
                if with_bias:
                    nc.tensor.matmul(pp, lhsT=b_t, rhs=ones_row, start=False, stop=True)
                nc.vector.tensor_copy(dst[:, r * NFREE : (r + 1) * NFREE], pp)

            def v_proj(b, r, half):
                for sub in (2 * half, 2 * half + 1):
                    kc = r * TPB + sub
                    pv = psmall.tile([P, DPC], F32, tag="ps", name=f"pv{b}_{kc}")
                    for c in range(EC):
                        nc.tensor.matmul(
                            pv,
                            lhsT=sl("e", b, r, c)[:, sub * P : (sub + 1) * P],
                            rhs=wv_sb[:, c],
                            start=(c == 0),
                            stop=(not with_bias and c == EC - 1),
                        )
                    if with_bias:
                        nc.tensor.matmul(
                            pv, lhsT=ones_row[:, :P], rhs=bv_sb, start=False, stop=True
                        )
                    for h in range(HPC):
                        nc.vector.tensor_copy(
                            v_sb[b][:, kc, h, 0:HD], pv[:, h * HD : (h + 1) * HD]
                        )

            # ---------- per-block ctx/norm/out tasks ----------
            pr_holder = {}
            T = _Task

            def make_ctx_tasks(bi):
                b, qc = ORDER[bi]
                base = NG * bi
                ctxps = {}
                ctxTs = {}
                ot = otp.tile([P, TPB, DPC], F32, tag="ot", name=f"ot{bi}")

                def ctx_chunk(h, j):
                    def run():
                        pr = pr_holder[bi]
                        if j == 0:
                            ctxps[h] = pctx.tile(
                                [HD + 1, NFREE], F32, tag="ctx", name=f"ctx{bi}_{h}"
                            )
                        cp = ctxps[h]
                        for kc in (2 * j, 2 * j + 1):
                            nc.tensor.matmul(
                                cp,
                                lhsT=v_sb[b][:, kc, h, :],
                                rhs=pr[:, 2 * kc + h, :],
                                start=(kc == 0),
                                stop=(kc == KC - 1),
                            )
                    return run

                def ctx_kc(h, kc):
                    def run():
                        pr = pr_holder[bi]
                        nc.tensor.matmul(
                            ctxps[h],
                            lhsT=v_sb[b][:, kc, h, :],
                            rhs=pr[:, 2 * kc + h, :],
                            start=False,
                            stop=(kc == KC - 1),
                        )
                    return run

                def drain(h):
                    def run():
                        ctxTs[h] = misc.tile(
                            [HD + 1, NFREE], BF16, tag="ctxT", name=f"ctxT{bi}_{h}"
                        )
                        nc.vector.tensor_copy(ctxTs[h], ctxps[h])
                    return run

                def norm(h, tlo=0, thi=TPB, ptag="tp"):
                    def run():
                        nt = thi - tlo
                        tp = psmall.tile(
                            [P, nt, 80], BF16, tag=ptag, name=f"tp{bi}_{h}_{tlo}"
                        )
                        for t in range(tlo, thi):
                            nc.tensor.transpose(
                                tp[:, t - tlo, 0 : HD + 1],
                                ctxTs[h][:, t * P : (t + 1) * P],
                                ident_bf[: HD + 1, : HD + 1],
                            )
                        rcp = misc.tile(
                            [P, nt, 1], F32, tag="rcp", bufs=4, name=f"rcp{bi}_{h}_{tlo}"
                        )
                        nc.vector.reciprocal(rcp, tp[:, :, HD : HD + 1])
                        nc.vector.tensor_mul(
                            ot[:, tlo:thi, h * HD : (h + 1) * HD],
                            tp[:, :, 0:HD],
                            rcp.broadcast_to([P, nt, HD]),
                        )
                    return run

                def dma_out(tlo, thi):
                    def run():
                        row0 = b * S + qc * NFREE + tlo * P
                        nc.sync.dma_start(
                            out.ap()[row0 : row0 + (thi - tlo) * P, :].rearrange(
                                "(t p) d -> p t d", p=P
                            ),
                            ot[:, tlo:thi, :],
                        )
                    return run

                tasks = []
                # b0's e-tiles (-> V kc8-15) only land ~G12-15, so block 0's
                # late ctx chunks shift into block 1's window; block 1 then
                # waits for block 0's pctx drains, block 2 half-recovers and
                # block 3+ runs the steady-state template.
                if bi == 0:
                    cg = [4, 6, 8, 10, 20, 22, 24, 26]
                    dr, nr, dm = 28, 30, 32
                elif bi == 1:
                    cg = [30, 31, 32, 33, 34, 35, 36, 37]
                    dr, nr, dm = 38, 40, 42
                elif bi == 2:
                    # j7 reads group 47's slots -> gate 48 (after exp(47))
                    cg = [40, 41, 42, 43, 44, 45, 46, 48]
                    dr, nr, dm = 49, 50, 52
                else:
                    cg = [base + 2 * j + 2 for j in range(8)]
                    dr, nr, dm = base + 17, base + 18, base + 20
                for j in range(8):
                    if bi == NB - 1 and j == 7:
                        # split the final chunk so only one 1-kc matmul per
                        # head remains after the last exp group
                        for h in range(HPC):
                            tasks.append(T(base + 15, base + 19, 215, ctx_kc(h, 14)))
                        for h in range(HPC):
                            tasks.append(T(base + 16, base + 20, 215, ctx_kc(h, 15)))
                        continue
                    for h in range(HPC):
                        tasks.append(T(cg[j] + h, cg[j] + h + 4, 430, ctx_chunk(h, j)))
                if bi < NB - 1:
                    for h in range(HPC):
                        tasks.append(T(dr + h, dr + h + 4, 100, drain(h)))
                        tasks.append(T(nr + h, nr + h + 5, 600, norm(h)))
                    tasks.append(T(dm, dm + 6, 0, dma_out(0, TPB)))
                else:
                    # fine-grained tail: 2-t norm chunks and split out-DMAs
                    # so the post-exp critical chain is as short as possible
                    tasks.append(T(base + 17, base + 23, 100, drain(0)))
                    tasks.append(T(base + 18, base + 24, 300, norm(0, 0, 2, "tp")))
                    tasks.append(T(base + 18, base + 25, 100, drain(1)))
                    tasks.append(T(base + 19, base + 25, 300, norm(0, 2, TPB, "ps")))
                    tasks.append(T(base + 19, base + 26, 300, norm(1, 0, 2, "tp")))
                    tasks.append(T(base + 20, base + 26, 0, dma_out(0, 2)))
                    tasks.append(T(base + 20, base + 27, 300, norm(1, 2, TPB, "ps")))
                    tasks.append(T(base + 21, base + 27, 0, dma_out(2, TPB)))
                return tasks

            # ---------- global filler task list ----------
            fillers = []

            def qk_task(b, r, which, gate, dl):
                fillers.append(
                    T(gate, dl, 1800, lambda b=b, r=r, w=which: qk_proj(b, r, w, psmall, "ps"))
                )

            def v_task(b, r, half, gate, dl):
                fillers.append(
                    T(gate, dl, 1040, lambda b=b, r=r, hf=half: v_proj(b, r, hf))
                )

            # b0 prep: e r1-3 proj feed score kc 4..15 (group == kc); gates
            # match expected DMA arrival so pops don't block the PE queue
            qk_task(0, 1, "e", 0, 4)
            qk_task(0, 2, "e", 4, 8)
            qk_task(0, 3, "e", 8, 12)
            v_task(0, 0, 0, 0, 4)
            v_task(0, 0, 1, 1, 6)
            v_task(0, 1, 0, 4, 8)
            v_task(0, 1, 1, 5, 10)
            v_task(0, 2, 0, 8, 20)
            v_task(0, 2, 1, 9, 22)
            v_task(0, 3, 0, 12, 24)
            v_task(0, 3, 1, 13, 26)
            qk_task(0, 1, "q", 6, 16)
            qk_task(0, 2, "q", 18, 32)
            qk_task(0, 3, "q", 30, 48)

            # b1 source DMA issues (esrc/qsrc rings are deep enough that
            # slot reuse only needs b0-r-readers of the same slot emitted)
            def dma_b1(which, r, eng):
                def run():
                    dma_src(which, 1, r, 0, 4, eng)
                    dma_src(which, 1, r, 4, 8, eng)
                return run

            for r in range(RC_B):
                fillers.append(T(12 + r, 30 + 4 * r, 0, dma_b1("e", r, nc.sync)))
            # q-ring slot reuse: q1r2/q1r3 land on q02/q03 slots, whose
            # readers qproj(0,2)/(0,3) are emitted by G30/G46 (deadlines).
            for r, gate in enumerate((17, 18, 31, 47)):
                fillers.append(T(gate, 48 + 4 * r, 0, dma_b1("q", r, nc.gpsimd)))

            # b1 prep — gated as early as the b1 DMAs plausibly land, with
            # deadlines a few groups before the scores that need them so the
            # work spreads over blocks 2-3 instead of bunching at G62-79
            qk_task(1, 0, "e", 28, 54)
            qk_task(1, 1, "e", 32, 56)
            qk_task(1, 2, "e", 36, 58)
            qk_task(1, 3, "e", 40, 60)
            qk_task(1, 0, "q", 42, 60)
            # vproj(1) is only needed from G66 — park it in the light back
            # half of the schedule instead of the oversubscribed front
            for r in range(RC_B):
                for half in (0, 1):
                    j = 2 * r + half
                    v_task(1, r, half, 50 + 2 * j, 66 + 2 * j)
            qk_task(1, 1, "q", 64, 78)
            qk_task(1, 2, "q", 72, 94)
            qk_task(1, 3, "q", 88, 110)

            for bi in range(NB):
                fillers += make_ctx_tasks(bi)

            fillers.sort(key=lambda t: (t.gate, t.deadline))

            # ---------- startup projections (r0 of b0) ----------
            # warm matmuls interleaved between the DMA-paced chunks keep the
            # PE array duty high so the HAM reaches 2.4 GHz by ~12us
            pp_e0 = spp.tile([P, NFREE], F32, tag="sp", name="pp_e00")
            pp_q0 = spp.tile([P, NFREE], F32, tag="sp", name="pp_q00")
            for c in range(EC):
                nc.tensor.matmul(
                    pp_e0, lhsT=wk_sb[:, c], rhs=sl("e", 0, 0, c),
                    start=(c == 0), stop=(not with_bias and c == EC - 1),
                )
                warm_mm(2)
            if with_bias:
                nc.tensor.matmul(pp_e0, lhsT=bk_sb, rhs=ones_row, start=False, stop=True)
            nc.vector.tensor_copy(kt_sb[0][:, 0:NFREE], pp_e0)
            for c in range(EC):
                nc.tensor.matmul(
                    pp_q0, lhsT=wq_sb[:, c], rhs=sl("q", 0, 0, c),
                    start=(c == 0), stop=(not with_bias and c == EC - 1),
                )
                warm_mm(1)
            if with_bias:
                nc.tensor.matmul(pp_q0, lhsT=bq_sb, rhs=ones_row, start=False, stop=True)
            nc.vector.tensor_copy(qt_sb[0][:, 0:NFREE], pp_q0)

            # ---------- flat attention stream ----------
            def scores_for(X):
                bi, g = divmod(X, NG)
                b, qc = ORDER[bi]
                col0 = qc * NFREE
                sp = spp.tile([P, HPC, NFREE], F32, tag="sp", name=f"sp{X}")
                for h in range(HPC):
                    d0 = h * HD
                    nc.tensor.matmul(
                        sp[:, h, :],
                        lhsT=kt_sb[b][d0 : d0 + HD, g * P : (g + 1) * P],
                        rhs=qt_sb[b][d0 : d0 + HD, col0 : col0 + NFREE],
                        start=True,
                        stop=True,
                    )
                return sp

            sps = {0: scores_for(0), 1: scores_for(1)}
            for G in range(NGT):
                bi, g = divmod(G, NG)
                if g == 0:
                    pr_holder[bi] = prp.tile(
                        [P, NSLOT, NFREE], BF16, tag="pr", name=f"pr{bi}"
                    )
                # forced pops: anything that must precede scores(G+2)
                i = 0
                while i < len(fillers):
                    if fillers[i].deadline <= G + 2:
                        fillers.pop(i).fn()
                    else:
                        i += 1
                # gate pops: gates encode both data arrival and load spread,
                # so pop everything that is ready
                while fillers and fillers[0].gate <= G:
                    fillers.pop(0).fn()
                if G + 2 < NGT:
                    sps[G + 2] = scores_for(G + 2)
                pr = pr_holder[bi]
                nc.scalar.activation(
                    pr[:, HPC * g : HPC * g + HPC, :], sps.pop(G), AF.Exp
                )
            while fillers:
                fillers.pop(0).fn()

    nc.finalize()
    return nc


def _get_nc(with_bias: bool = True) -> bass.Bass:
    if with_bias not in _CACHED_NC:
        _CACHED_NC[with_bias] = _build_nc(with_bias)
    return _CACHED_NC[with_bias]


def kernel(embed, q, Wk, bk, Wq, bq, Wv, bv, trace=False):
    global LAST_RESULTS
    bf = ml_dtypes.bfloat16
    embed = np.asarray(embed, dtype=np.float32)
    q = np.asarray(q, dtype=np.float32)
    Wk = np.asarray(Wk, dtype=np.float32)
    Wq = np.asarray(Wq, dtype=np.float32)
    Wv = np.asarray(Wv, dtype=np.float32)
    bk = np.asarray(bk, dtype=np.float32)
    bq = np.asarray(bq, dtype=np.float32)
    bv = np.asarray(bv, dtype=np.float32)

    qTt = _tile_inputs(q.reshape(ROWS, E))
    eTt = _tile_inputs(embed.reshape(ROWS, E))

    in_maps = []
    for c in range(NCORES):
        sl = slice(c * DPC, (c + 1) * DPC)
        in_maps.append(
            {
                "qTt": qTt,
                "eTt": eTt,
                # scores scale folded into Wq/bq (exact: *2^-3)
                "WqT": _tile_w(np.ascontiguousarray((Wq[sl] * SCALE).T).astype(bf)),
                "WkT": _tile_w(np.ascontiguousarray(Wk[sl].T).astype(bf)),
                "WvT": _tile_w(np.ascontiguousarray(Wv[sl].T).astype(bf)),
                "bqs": (bq[sl] * SCALE).astype(bf),
                "bkp": bk[sl].astype(bf),
                "bvp": bv[sl].astype(bf),
                "idn": np.eye(P, dtype=np.float32).astype(bf),
            }
        )

    with_bias = bool(bq.any() or bk.any() or bv.any())
    nc = _get_nc(with_bias)
    res = run_bass_kernel_spmd(nc, in_maps, list(range(NCORES)), trace=trace)
    LAST_RESULTS = res

    full = np.empty((ROWS, E), dtype=np.float32)
    for c in range(NCORES):
        full[:, c * DPC : (c + 1) * DPC] = res.results[c]["out"]
    return full.reshape(B, S, E)


# revision 36
# speedup vs baseline: 1.0393x; 1.0096x over previous
"""Cross multi-head attention TRN2 kernel (8-core SPMD, head-sharded), v2.

Strategy (tensor parallel over heads, zero communication):
  - 16 heads / 8 cores -> 2 heads per core. Core c computes output columns
    [128*c, 128*(c+1)) of the [4096, 1024] output; host concatenates.
  - Host pre-transposes and PRE-TILES q/embed into [tile, P, chunks] bf16
    layouts so every input DMA is contiguous per partition.
  - Scores are computed transposed (S^T[k, q] = K.Q^T, scale folded into Wq).
    The two heads per core live on partition halves 0-63 / 64-127, so their
    K=64 score matmuls row-tile and overlap ~2x on the PE.
  - The exp stream is 128 uniform groups of 2 score slots (one kc, both
    heads) x [P, 2, 512] fp32 PSUM, double buffered (4 banks); ACT is the
    near-critical engine (~130us) and the PE (~140us) paces the kernel.
  - All other PE work (attn@V 2-kc chunks, projections, V-proj, ctx
    transposes) is a task list with (gate, deadline) bounds popped between
    exp groups; deadlines are tight so work never bursts at block edges.
  - Softmax denominator via a ones-column appended to V (attn@V also
    produces row-sums); ctx'^T is PE-transposed back to [q, d] (bf16),
    normalized per-partition (DVE reciprocal+mul), DMA'd out per block.
"""

import numpy as np
import ml_dtypes

import concourse.bass as bass
import concourse.bacc as bacc
import concourse.mybir as mybir
import concourse.tile as tile
from concourse.bass_utils import run_bass_kernel_spmd

# ---- problem dims (hardcoded; kernel.py must be self-contained) ----
B, S, E = 2, 2048, 1024
NHEAD, HD = 16, 64
NCORES = 8
HPC = NHEAD // NCORES          # heads per core = 2
DPC = HPC * HD                 # projection out-dims per core = 128
ROWS = B * S                   # 4096
P = 128                        # SBUF partitions
NFREE = 512                    # matmul moving free dim (one PSUM bank fp32)
EC = E // P                    # 8 contraction chunks
KC = S // P                    # 16 key chunks per batch
QC = S // NFREE                # 4 query chunks per batch
RC_B = S // NFREE              # 4 projection row-chunks per batch
TPB = NFREE // P               # 4 transpose chunks per block
NSLOT = HPC * KC               # 32 score slots per (b,qc) block
NG = KC                        # 16 exp groups per block (2 slots each)
NB = B * QC                    # 8 blocks
NGT = NB * NG                  # 128 groups total
SCALE = 1.0 / np.sqrt(HD)      # 0.125, folded into Wq/bq on host

F32 = mybir.dt.float32
BF16 = mybir.dt.bfloat16
AF = mybir.ActivationFunctionType

_CACHED_NC = {}
LAST_RESULTS = None            # test.py reads exec_time_ns / profile from here

ORDER = [(0, 0), (0, 1), (0, 2), (0, 3), (1, 0), (1, 1), (1, 2), (1, 3)]


def _tile_inputs(mat_rows_e: np.ndarray) -> np.ndarray:
    """[ROWS, E] f32 -> pre-tiled bf16 [P, B*RC_B, EC*NFREE]: slice
    [:, b*RC_B+r, :] is one projection row-chunk, 8KB contiguous per
    partition, laid out [c, n] with E-index = c*128 + p, col = b*S +
    r*512 + n."""
    t = np.ascontiguousarray(mat_rows_e.T).astype(ml_dtypes.bfloat16)
    a = t.reshape(EC, P, B, RC_B, NFREE)            # [c, p, b, r, n]
    a = a.transpose(1, 2, 3, 0, 4)                  # [p, b, r, c, n]
    return np.ascontiguousarray(a.reshape(P, B * RC_B, EC * NFREE))


def _tile_w(wT: np.ndarray) -> np.ndarray:
    """[E, DPC] bf16 weight -> [P, EC*DPC]: contiguous per partition."""
    a = wT.reshape(EC, P, DPC).transpose(1, 0, 2)
    return np.ascontiguousarray(a.reshape(P, EC * DPC))


class _Task:
    """Filler work item: gate = earliest group index after whose exp it may
    be emitted; deadline = group whose SCORES it must precede (forced-pop
    at G >= deadline-2); cost = PE-ns estimate for pacing."""

    __slots__ = ("gate", "deadline", "cost", "fn")

    def __init__(self, gate, deadline, cost, fn):
        self.gate = gate
        self.deadline = deadline
        self.cost = cost
        self.fn = fn


def _build_nc(with_bias: bool) -> bass.Bass:
    nc = bacc.Bacc(
        "TRN2",
        target_bir_lowering=False,
        debug=False,
        num_devices=NCORES,
    )

    qTt = nc.declare_dram_parameter("qTt", [P, B * RC_B, EC * NFREE], BF16, isOutput=False)
    eTt = nc.declare_dram_parameter("eTt", [P, B * RC_B, EC * NFREE], BF16, isOutput=False)
    WqT = nc.declare_dram_parameter("WqT", [P, EC * DPC], BF16, isOutput=False)
    WkT = nc.declare_dram_parameter("WkT", [P, EC * DPC], BF16, isOutput=False)
    WvT = nc.declare_dram_parameter("WvT", [P, EC * DPC], BF16, isOutput=False)
    bqs = nc.declare_dram_parameter("bqs", [DPC], BF16, isOutput=False)
    bkp = nc.declare_dram_parameter("bkp", [DPC], BF16, isOutput=False)
    bvp = nc.declare_dram_parameter("bvp", [DPC], BF16, isOutput=False)
    idn = nc.declare_dram_parameter("idn", [P, P], BF16, isOutput=False)
    out = nc.declare_dram_parameter("out", [ROWS, DPC], F32, isOutput=True)

    with tile.TileContext(nc) as tc:
        with (
            tc.tile_pool(name="consts", bufs=1) as consts,
            tc.tile_pool(name="wpool", bufs=1) as wpool,
            tc.tile_pool(name="resid", bufs=1) as resid,
            tc.tile_pool(name="esrc", bufs=1) as esrc,
            tc.tile_pool(name="qsrc", bufs=1) as qsrc,
            tc.tile_pool(name="prp", bufs=2) as prp,
            tc.tile_pool(name="misc", bufs=2) as misc,
            tc.tile_pool(name="otp", bufs=4) as otp,
            # PSUM banks: sp 2x2 + ctx 2 + ps 1 + tp 1 = 8
            tc.tile_pool(name="spp", bufs=2, space="PSUM") as spp,
            tc.tile_pool(name="pctx", bufs=2, space="PSUM") as pctx,
            tc.tile_pool(name="psmall", bufs=1, space="PSUM") as psmall,
        ):
            # ---------- weights first (first c-chunks split out so the
            # very first projection matmuls can start early) ----
            wk_sb = wpool.tile([P, EC, DPC], BF16, name="wk_sb")
            nc.sync.dma_start(
                wk_sb[:, 0:2], WkT.ap()[:, : 2 * DPC].rearrange("p (c d) -> p c d", c=2)
            )
            # identity via DMA (host-prepared): ready ~9us, feeds the PE
            # warm-up chain — gpsimd make_identity would land far too late
            ident_bf = consts.tile([P, P], BF16, name="ident_bf")
            nc.scalar.dma_start(ident_bf, idn.ap())
            wq_sb = wpool.tile([P, EC, DPC], BF16, name="wq_sb")
            nc.scalar.dma_start(
                wq_sb[:, 0:2], WqT.ap()[:, : 2 * DPC].rearrange("p (c d) -> p c d", c=2)
            )
            nc.sync.dma_start(
                wk_sb[:, 2:], WkT.ap()[:, 2 * DPC :].rearrange("p (c d) -> p c d", c=EC - 2)
            )
            nc.scalar.dma_start(
                wq_sb[:, 2:], WqT.ap()[:, 2 * DPC :].rearrange("p (c d) -> p c d", c=EC - 2)
            )

            # ---------- source DMAs (chunked along E-contraction) --------
            src_chunks = {}

            def dma_src(which, b, r, lo, hi, eng):
                dram = qTt if which == "q" else eTt
                tag = f"{which}{hi - lo}"
                tl = (qsrc if which == "q" else esrc).tile(
                    [P, hi - lo, NFREE], BF16, tag=tag, bufs=(4 if hi - lo == 2 else 8),
                    name=f"{which}{b}{r}c{lo}",
                )
                eng.dma_start(
                    tl,
                    dram.ap()[:, b * RC_B + r, lo * NFREE : hi * NFREE].rearrange(
                        "p (c n) -> p c n", c=hi - lo
                    ),
                )
                src_chunks.setdefault((which, b, r), []).append((tl, lo, hi))

            def sl(which, b, r, c):
                for tl, lo, hi in src_chunks[(which, b, r)]:
                    if lo <= c < hi:
                        return tl[:, c - lo]
                raise KeyError((which, b, r, c))

            # startup-critical quarters: e00 on sync+scalar, q00 on
            # gpsimd+scalar (3 DMA-capable rings: sync, scalar, gpsimd)
            # ~0.95MB per ring so kproj+qproj inputs all land ~16-17.5us
            dma_src("e", 0, 0, 0, 2, nc.sync)
            dma_src("e", 0, 0, 4, 6, nc.scalar)
            dma_src("q", 0, 0, 0, 2, nc.gpsimd)
            dma_src("q", 0, 0, 4, 6, nc.sync)
            dma_src("e", 0, 0, 6, 8, nc.scalar)
            dma_src("q", 0, 0, 2, 4, nc.gpsimd)
            dma_src("q", 0, 0, 6, 8, nc.sync)
            dma_src("e", 0, 0, 2, 4, nc.gpsimd)
            # wv + biases on gpsimd (needed by vproj from G0ish — must be
            # queued ahead of the bulky q01-03 halves)
            wv_sb = wpool.tile([P, EC, DPC], BF16, name="wv_sb")
            nc.gpsimd.dma_start(wv_sb, WvT.ap().rearrange("p (c d) -> p c d", c=EC))
            bq_sb = wpool.tile([1, DPC], BF16, name="bq_sb")
            nc.gpsimd.dma_start(bq_sb, bqs.ap()[None, :])
            bk_sb = wpool.tile([1, DPC], BF16, name="bk_sb")
            nc.gpsimd.dma_start(bk_sb, bkp.ap()[None, :])
            bv_sb = wpool.tile([1, DPC], BF16, name="bv_sb")
            nc.gpsimd.dma_start(bv_sb, bvp.ap()[None, :])
            # b0 e-halves spread over all three rings, ordered by need time
            # (e01 by ~G2, e02 by ~G6, e03 by ~G10)
            dma_src("e", 0, 1, 0, 4, nc.sync)
            dma_src("e", 0, 1, 4, 8, nc.scalar)
            dma_src("e", 0, 2, 0, 4, nc.gpsimd)
            dma_src("e", 0, 2, 4, 8, nc.scalar)
            dma_src("e", 0, 3, 0, 4, nc.sync)
            dma_src("e", 0, 3, 4, 8, nc.gpsimd)
            for r in (1, 2, 3):
                dma_src("q", 0, r, 0, 4, nc.gpsimd)
                dma_src("q", 0, r, 4, 8, nc.gpsimd)

            # ---------- constants ----------
            ones_row = consts.tile([1, NFREE], BF16)
            nc.vector.memset(ones_row, 1.0)
            # warm the PE HAM (flip to 2.4 GHz) during the initial DMA wait.
            # K=128 dense matmuls on the identity (K=1 or tiny-N chains keep
            # the array duty too low to trip the HAM busy window).
            wmp = psmall.tile([P, NFREE], F32, tag="ps", name="wmp")

            def warm_mm(n):
                for _ in range(n):
                    nc.tensor.matmul(
                        wmp[:, 0:P], lhsT=ident_bf, rhs=ident_bf,
                        start=True, stop=True,
                    )

            warm_mm(24)
            # warm the ACT exp table (after scalar's DMA issues)
            warm = consts.tile([1, 1], BF16)
            nc.scalar.activation(warm, ones_row[:, 0:1], AF.Exp)

            # ---------- residents (per batch) ----------
            qt_sb = []
            kt_sb = []
            v_sb = []
            for b in range(B):
                qt = resid.tile([P, S], BF16, name=f"qt{b}")
                kt = resid.tile([P, S], BF16, name=f"kt{b}")
                vv = resid.tile([P, KC, HPC, HD + 1], BF16, name=f"v{b}")
                nc.vector.memset(vv[:, :, :, HD : HD + 1], 1.0)
                qt_sb.append(qt)
                kt_sb.append(kt)
                v_sb.append(vv)

            # ---------- projections ----------
            def qk_proj(b, r, which, pool_, tag):
                if which == "q":
                    w_t, b_t, dst = wq_sb, bq_sb, qt_sb[b]
                else:
                    w_t, b_t, dst = wk_sb, bk_sb, kt_sb[b]
                pp = pool_.tile([P, NFREE], F32, tag=tag, name=f"pp{which}{b}_{r}")
                for c in range(EC):
                    nc.tensor.matmul(
                        pp,
                        lhsT=w_t[:, c],
                        rhs=sl(which, b, r, c),
                        start=(c == 0),
                        stop=(not with_bias and c == EC - 1),
                    )
                if with_bias:
                    nc.tensor.matmul(pp, lhsT=b_t, rhs=ones_row, start=False, stop=True)
                nc.vector.tensor_copy(dst[:, r * NFREE : (r + 1) * NFREE], pp)

            def v_proj(b, r, half):
                for sub in (2 * half, 2 * half + 1):
                    kc = r * TPB + sub
                    pv = psmall.tile([P, DPC], F32, tag="ps", name=f"pv{b}_{kc}")
                    for c in range(EC):
                        nc.tensor.matmul(
                            pv,
                            lhsT=sl("e", b, r, c)[:, sub * P : (sub + 1) * P],
                            rhs=wv_sb[:, c],
                            start=(c == 0),
                            stop=(not with_bias and c == EC - 1),
                        )
                    if with_bias:
                        nc.tensor.matmul(
                            pv, lhsT=ones_row[:, :P], rhs=bv_sb, start=False, stop=True
                        )
                    for h in range(HPC):
                        nc.vector.tensor_copy(
                            v_sb[b][:, kc, h, 0:HD], pv[:, h * HD : (h + 1) * HD]
                        )

            # ---------- per-block ctx/norm/out tasks ----------
            pr_holder = {}
            T = _Task

            def make_ctx_tasks(bi):
                b, qc = ORDER[bi]
                base = NG * bi
                ctxps = {}
                ctxTs = {}
                ot = otp.tile([P, TPB, DPC], F32, tag="ot", name=f"ot{bi}")

                def ctx_chunk(h, j):
                    def run():
                        pr = pr_holder[bi]
                        if j == 0:
                            ctxps[h] = pctx.tile(
                                [HD + 1, NFREE], F32, tag="ctx", name=f"ctx{bi}_{h}"
                            )
                        cp = ctxps[h]
                        for kc in (2 * j, 2 * j + 1):
                            nc.tensor.matmul(
                                cp,
                                lhsT=v_sb[b][:, kc, h, :],
                                rhs=pr[:, 2 * kc + h, :],
                                start=(kc == 0),
                                stop=(kc == KC - 1),
                            )
                    return run

                def ctx_kc(h, kc):
                    def run():
                        pr = pr_holder[bi]
                        nc.tensor.matmul(
                            ctxps[h],
                            lhsT=v_sb[b][:, kc, h, :],
                            rhs=pr[:, 2 * kc + h, :],
                            start=False,
                            stop=(kc == KC - 1),
                        )
                    return run

                def drain(h):
                    def run():
                        ctxTs[h] = misc.tile(
                            [HD + 1, NFREE], BF16, tag="ctxT", name=f"ctxT{bi}_{h}"
                        )
                        nc.vector.tensor_copy(ctxTs[h], ctxps[h])
                    return run

                def norm(h, tlo=0, thi=TPB, ptag="tp"):
                    def run():
                        nt = thi - tlo
                        tp = psmall.tile(
                            [P, nt, 80], BF16, tag=ptag, name=f"tp{bi}_{h}_{tlo}"
                        )
                        for t in range(tlo, thi):
                            nc.tensor.transpose(
                                tp[:, t - tlo, 0 : HD + 1],
                                ctxTs[h][:, t * P : (t + 1) * P],
                                ident_bf[: HD + 1, : HD + 1],
                            )
                        rcp = misc.tile(
                            [P, nt, 1], F32, tag="rcp", bufs=4, name=f"rcp{bi}_{h}_{tlo}"
                        )
                        nc.vector.reciprocal(rcp, tp[:, :, HD : HD + 1])
                        nc.vector.tensor_mul(
                            ot[:, tlo:thi, h * HD : (h + 1) * HD],
                            tp[:, :, 0:HD],
                            rcp.broadcast_to([P, nt, HD]),
                        )
                    return run

                def dma_out(tlo, thi):
                    def run():
                        row0 = b * S + qc * NFREE + tlo * P
                        nc.sync.dma_start(
                            out.ap()[row0 : row0 + (thi - tlo) * P, :].rearrange(
                                "(t p) d -> p t d", p=P
                            ),
                            ot[:, tlo:thi, :],
                        )
                    return run

                tasks = []
                # b0's e-tiles (-> V kc8-15) only land ~G12-15, so block 0's
                # late ctx chunks shift into block 1's window; block 1 then
                # waits for block 0's pctx drains, block 2 half-recovers and
                # block 3+ runs the steady-state template.
                if bi == 0:
                    cg = [4, 6, 8, 10, 20, 22, 24, 26]
                    dr, nr, dm = 28, 30, 32
                elif bi == 1:
                    cg = [30, 31, 32, 33, 34, 35, 36, 37]
                    dr, nr, dm = 38, 40, 42
                elif bi == 2:
                    # j7 reads group 47's slots -> gate 48 (after exp(47))
                    cg = [40, 41, 42, 43, 44, 45, 46, 48]
                    dr, nr, dm = 49, 50, 52
                else:
                    cg = [base + 2 * j + 2 for j in range(8)]
                    dr, nr, dm = base + 17, base + 18, base + 20
                for j in range(8):
                    if bi == NB - 1 and j == 7:
                        # split the final chunk so only one 1-kc matmul per
                        # head remains after the last exp group
                        for h in range(HPC):
                            tasks.append(T(base + 15, base + 19, 215, ctx_kc(h, 14)))
                        for h in range(HPC):
                            tasks.append(T(base + 16, base + 20, 215, ctx_kc(h, 15)))
                        continue
                    for h in range(HPC):
                        tasks.append(T(cg[j] + h, cg[j] + h + 4, 430, ctx_chunk(h, j)))
                if bi < NB - 1:
                    for h in range(HPC):
                        tasks.append(T(dr + h, dr + h + 4, 100, drain(h)))
                        tasks.append(T(nr + h, nr + h + 5, 600, norm(h)))
                    tasks.append(T(dm, dm + 6, 0, dma_out(0, TPB)))
                else:
                    # fine-grained tail: 2-t norm chunks and split out-DMAs
                    # so the post-exp critical chain is as short as possible
                    tasks.append(T(base + 17, base + 23, 100, drain(0)))
                    tasks.append(T(base + 18, base + 24, 300, norm(0, 0, 2, "tp")))
                    tasks.append(T(base + 18, base + 25, 100, drain(1)))
                    tasks.append(T(base + 19, base + 25, 300, norm(0, 2, TPB, "ps")))
                    tasks.append(T(base + 19, base + 26, 300, norm(1, 0, 2, "tp")))
                    tasks.append(T(base + 20, base + 26, 0, dma_out(0, 2)))
                    tasks.append(T(base + 20, base + 27, 300, norm(1, 2, TPB, "ps")))
                    tasks.append(T(base + 21, base + 27, 0, dma_out(2, TPB)))
                return tasks

            # ---------- global filler task list ----------
            fillers = []

            def qk_task(b, r, which, gate, dl):
                fillers.append(
                    T(gate, dl, 1800, lambda b=b, r=r, w=which: qk_proj(b, r, w, psmall, "ps"))
                )

            def v_task(b, r, half, gate, dl):
                fillers.append(
                    T(gate, dl, 1040, lambda b=b, r=r, hf=half: v_proj(b, r, hf))
                )

            # b0 prep: e r1-3 proj feed score kc 4..15 (group == kc); gates
            # match expected DMA arrival so pops don't block the PE queue
            qk_task(0, 1, "e", 0, 4)
            qk_task(0, 2, "e", 4, 8)
            qk_task(0, 3, "e", 8, 12)
            v_task(0, 0, 0, 0, 4)
            v_task(0, 0, 1, 1, 6)
            v_task(0, 1, 0, 4, 8)
            v_task(0, 1, 1, 5, 10)
            v_task(0, 2, 0, 8, 20)
            v_task(0, 2, 1, 9, 22)
            v_task(0, 3, 0, 12, 24)
            v_task(0, 3, 1, 13, 26)
            qk_task(0, 1, "q", 6, 16)
            qk_task(0, 2, "q", 18, 32)
            qk_task(0, 3, "q", 30, 48)

            # b1 source DMA issues (esrc/qsrc rings are deep enough that
            # slot reuse only needs b0-r-readers of the same slot emitted)
            def dma_b1(which, r, eng):
                def run():
                    dma_src(which, 1, r, 0, 4, eng)
                    dma_src(which, 1, r, 4, 8, eng)
                return run

            for r in range(RC_B):
                fillers.append(T(12 + r, 30 + 4 * r, 0, dma_b1("e", r, nc.sync)))
            # q-ring slot reuse: q1r2/q1r3 land on q02/q03 slots, whose
            # readers qproj(0,2)/(0,3) are emitted by G30/G46 (deadlines).
            for r, gate in enumerate((17, 18, 31, 47)):
                fillers.append(T(gate, 48 + 4 * r, 0, dma_b1("q", r, nc.gpsimd)))

            # b1 prep — gated as early as the b1 DMAs plausibly land, with
            # deadlines a few groups before the scores that need them so the
            # work spreads over blocks 2-3 instead of bunching at G62-79
            qk_task(1, 0, "e", 28, 54)
            qk_task(1, 1, "e", 32, 56)
            qk_task(1, 2, "e", 36, 58)
            qk_task(1, 3, "e", 40, 60)
            qk_task(1, 0, "q", 42, 60)
            # vproj(1) is only needed from G66 — park it in the light back
            # half of the schedule instead of the oversubscribed front
            for r in range(RC_B):
                for half in (0, 1):
                    j = 2 * r + half
                    v_task(1, r, half, 50 + 2 * j, 66 + 2 * j)
            qk_task(1, 1, "q", 64, 78)
            qk_task(1, 2, "q", 72, 94)
            qk_task(1, 3, "q", 88, 110)

            for bi in range(NB):
                fillers += make_ctx_tasks(bi)

            fillers.sort(key=lambda t: (t.gate, t.deadline))

            # ---------- startup projections (r0 of b0) ----------
            # warm matmuls interleaved between the DMA-paced chunks keep the
            # PE array duty high so the HAM reaches 2.4 GHz by ~12us
            pp_e0 = spp.tile([P, NFREE], F32, tag="sp", name="pp_e00")
            pp_q0 = spp.tile([P, NFREE], F32, tag="sp", name="pp_q00")
            for c in range(EC):
                nc.tensor.matmul(
                    pp_e0, lhsT=wk_sb[:, c], rhs=sl("e", 0, 0, c),
                    start=(c == 0), stop=(not with_bias and c == EC - 1),
                )
                warm_mm(2)
            if with_bias:
                nc.tensor.matmul(pp_e0, lhsT=bk_sb, rhs=ones_row, start=False, stop=True)
            nc.vector.tensor_copy(kt_sb[0][:, 0:NFREE], pp_e0)
            for c in range(EC):
                nc.tensor.matmul(
                    pp_q0, lhsT=wq_sb[:, c], rhs=sl("q", 0, 0, c),
                    start=(c == 0), stop=(not with_bias and c == EC - 1),
                )
                warm_mm(1)
            if with_bias:
                nc.tensor.matmul(pp_q0, lhsT=bq_sb, rhs=ones_row, start=False, stop=True)
            nc.vector.tensor_copy(qt_sb[0][:, 0:NFREE], pp_q0)

            # ---------- flat attention stream ----------
            def scores_for(X):
                bi, g = divmod(X, NG)
                b, qc = ORDER[bi]
                col0 = qc * NFREE
                sp = spp.tile([P, HPC, NFREE], F32, tag="sp", name=f"sp{X}")
                for h in range(HPC):
                    d0 = h * HD
                    nc.tensor.matmul(
                        sp[:, h, :],
                        lhsT=kt_sb[b][d0 : d0 + HD, g * P : (g + 1) * P],
                        rhs=qt_sb[b][d0 : d0 + HD, col0 : col0 + NFREE],
                        start=True,
                        stop=True,
                    )
                return sp

            sps = {0: scores_for(0), 1: scores_for(1)}
            for G in range(NGT):
                bi, g = divmod(G, NG)
                if g == 0:
                    pr_holder[bi] = prp.tile(
                        [P, NSLOT, NFREE], BF16, tag="pr", name=f"pr{bi}"
                    )
                # forced pops: anything that must precede scores(G+2)
                i = 0
                while i < len(fillers):
                    if fillers[i].deadline <= G + 2:
                        fillers.pop(i).fn()
                    else:
                        i += 1
                # gate pops: gates encode both data arrival and load spread,
                # so pop everything that is ready
                while fillers and fillers[0].gate <= G:
                    fillers.pop(0).fn()
                if G + 2 < NGT:
                    sps[G + 2] = scores_for(G + 2)
                pr = pr_holder[bi]
                nc.scalar.activation(
                    pr[:, HPC * g : HPC * g + HPC, :], sps.pop(G), AF.Exp
                )
            while fillers:
                fillers.pop(0).fn()

    nc.finalize()
    return nc


def _get_nc(with_bias: bool = True) -> bass.Bass:
    if with_bias not in _CACHED_NC:
        _CACHED_NC[with_bias] = _build_nc(with_bias)
    return _CACHED_NC[with_bias]


def kernel(embed, q, Wk, bk, Wq, bq, Wv, bv, trace=False):
    global LAST_RESULTS
    bf = ml_dtypes.bfloat16
    embed = np.asarray(embed, dtype=np.float32)
    q = np.asarray(q, dtype=np.float32)
    Wk = np.asarray(Wk, dtype=np.float32)
    Wq = np.asarray(Wq, dtype=np.float32)
    Wv = np.asarray(Wv, dtype=np.float32)
    bk = np.asarray(bk, dtype=np.float32)
    bq = np.asarray(bq, dtype=np.float32)
    bv = np.asarray(bv, dtype=np.float32)

    qTt = _tile_inputs(q.reshape(ROWS, E))
    eTt = _tile_inputs(embed.reshape(ROWS, E))

    in_maps = []
    for c in range(NCORES):
        sl = slice(c * DPC, (c + 1) * DPC)
        in_maps.append(
            {
                "qTt": qTt,
                "eTt": eTt,
                # scores scale folded into Wq/bq (exact: *2^-3)
                "WqT": _tile_w(np.ascontiguousarray((Wq[sl] * SCALE).T).astype(bf)),
                "WkT": _tile_w(np.ascontiguousarray(Wk[sl].T).astype(bf)),
                "WvT": _tile_w(np.ascontiguousarray(Wv[sl].T).astype(bf)),
                "bqs": (bq[sl] * SCALE).astype(bf),
                "bkp": bk[sl].astype(bf),
                "bvp": bv[sl].astype(bf),
                "idn": np.eye(P, dtype=np.float32).astype(bf),
            }
        )

    with_bias = bool(bq.any() or bk.any() or bv.any())
    nc = _get_nc(with_bias)
    res = run_bass_kernel_spmd(nc, in_maps, list(range(NCORES)), trace=trace)
    LAST_RESULTS = res

    full = np.empty((ROWS, E), dtype=np.float32)
    for c in range(NCORES):
        full[:, c * DPC : (c + 1) * DPC] = res.results[c]["out"]
    return full.reshape(B, S, E)


# revision 41
# speedup vs baseline: 1.0637x; 1.0235x over previous
"""Cross multi-head attention TRN2 kernel (8-core SPMD, head-sharded), v2.

Strategy (tensor parallel over heads, zero communication):
  - 16 heads / 8 cores -> 2 heads per core. Core c computes output columns
    [128*c, 128*(c+1)) of the [4096, 1024] output; host concatenates.
  - Host pre-transposes and PRE-TILES q/embed into [tile, P, chunks] bf16
    layouts so every input DMA is contiguous per partition.
  - Scores are computed transposed (S^T[k, q] = K.Q^T, scale folded into Wq).
    The two heads per core live on partition halves 0-63 / 64-127, so their
    K=64 score matmuls row-tile and overlap ~2x on the PE.
  - The exp stream is 128 uniform groups of 2 score slots (one kc, both
    heads) x [P, 2, 512] fp32 PSUM, double buffered (4 banks); ACT is the
    near-critical engine (~130us) and the PE (~140us) paces the kernel.
  - All other PE work (attn@V 2-kc chunks, projections, V-proj, ctx
    transposes) is a task list with (gate, deadline) bounds popped between
    exp groups; deadlines are tight so work never bursts at block edges.
  - Softmax denominator via a ones-column appended to V (attn@V also
    produces row-sums); ctx'^T is PE-transposed back to [q, d] (bf16),
    normalized per-partition (DVE reciprocal+mul), DMA'd out per block.
"""

import numpy as np
import ml_dtypes

import concourse.bass as bass
import concourse.bacc as bacc
import concourse.mybir as mybir
import concourse.tile as tile
from concourse.bass_utils import run_bass_kernel_spmd

# ---- problem dims (hardcoded; kernel.py must be self-contained) ----
B, S, E = 2, 2048, 1024
NHEAD, HD = 16, 64
NCORES = 8
HPC = NHEAD // NCORES          # heads per core = 2
DPC = HPC * HD                 # projection out-dims per core = 128
ROWS = B * S                   # 4096
P = 128                        # SBUF partitions
NFREE = 512                    # matmul moving free dim (one PSUM bank fp32)
EC = E // P                    # 8 contraction chunks
KC = S // P                    # 16 key chunks per batch
QC = S // NFREE                # 4 query chunks per batch
RC_B = S // NFREE              # 4 projection row-chunks per batch
TPB = NFREE // P               # 4 transpose chunks per block
NSLOT = HPC * KC               # 32 score slots per (b,qc) block
NG = KC                        # 16 exp groups per block (2 slots each)
NB = B * QC                    # 8 blocks
NGT = NB * NG                  # 128 groups total
SCALE = 1.0 / np.sqrt(HD)      # 0.125, folded into Wq/bq on host

F32 = mybir.dt.float32
BF16 = mybir.dt.bfloat16
AF = mybir.ActivationFunctionType

_CACHED_NC = {}
LAST_RESULTS = None            # test.py reads exec_time_ns / profile from here

ORDER = [(0, 0), (0, 1), (0, 2), (0, 3), (1, 0), (1, 1), (1, 2), (1, 3)]


def _tile_inputs(mat_rows_e: np.ndarray) -> np.ndarray:
    """[ROWS, E] f32 -> pre-tiled bf16 [P, B*RC_B, EC*NFREE]: slice
    [:, b*RC_B+r, :] is one projection row-chunk, 8KB contiguous per
    partition, laid out [c, n] with E-index = c*128 + p, col = b*S +
    r*512 + n."""
    t = np.ascontiguousarray(mat_rows_e.T).astype(ml_dtypes.bfloat16)
    a = t.reshape(EC, P, B, RC_B, NFREE)            # [c, p, b, r, n]
    a = a.transpose(1, 2, 3, 0, 4)                  # [p, b, r, c, n]
    return np.ascontiguousarray(a.reshape(P, B * RC_B, EC * NFREE))


def _tile_w(wT: np.ndarray) -> np.ndarray:
    """[E, DPC] bf16 weight -> [P, EC*DPC]: contiguous per partition."""
    a = wT.reshape(EC, P, DPC).transpose(1, 0, 2)
    return np.ascontiguousarray(a.reshape(P, EC * DPC))


class _Task:
    """Filler work item: gate = earliest group index after whose exp it may
    be emitted; deadline = group whose SCORES it must precede (forced-pop
    at G >= deadline-2); cost = PE-ns estimate for pacing."""

    __slots__ = ("gate", "deadline", "cost", "fn")

    def __init__(self, gate, deadline, cost, fn):
        self.gate = gate
        self.deadline = deadline
        self.cost = cost
        self.fn = fn


def _build_nc(with_bias: bool) -> bass.Bass:
    nc = bacc.Bacc(
        "TRN2",
        target_bir_lowering=False,
        debug=False,
        num_devices=NCORES,
    )

    qTt = nc.declare_dram_parameter("qTt", [P, B * RC_B, EC * NFREE], BF16, isOutput=False)
    eTt = nc.declare_dram_parameter("eTt", [P, B * RC_B, EC * NFREE], BF16, isOutput=False)
    WqT = nc.declare_dram_parameter("WqT", [P, EC * DPC], BF16, isOutput=False)
    WkT = nc.declare_dram_parameter("WkT", [P, EC * DPC], BF16, isOutput=False)
    WvT = nc.declare_dram_parameter("WvT", [P, EC * DPC], BF16, isOutput=False)
    bqs = nc.declare_dram_parameter("bqs", [DPC], BF16, isOutput=False)
    bkp = nc.declare_dram_parameter("bkp", [DPC], BF16, isOutput=False)
    bvp = nc.declare_dram_parameter("bvp", [DPC], BF16, isOutput=False)
    idn = nc.declare_dram_parameter("idn", [P, P], BF16, isOutput=False)
    out = nc.declare_dram_parameter("out", [ROWS, DPC], F32, isOutput=True)

    with tile.TileContext(nc) as tc:
        with (
            tc.tile_pool(name="consts", bufs=1) as consts,
            tc.tile_pool(name="wpool", bufs=1) as wpool,
            tc.tile_pool(name="resid", bufs=1) as resid,
            tc.tile_pool(name="esrc", bufs=1) as esrc,
            tc.tile_pool(name="qsrc", bufs=1) as qsrc,
            tc.tile_pool(name="prp", bufs=2) as prp,
            tc.tile_pool(name="misc", bufs=2) as misc,
            tc.tile_pool(name="otp", bufs=4) as otp,
            # PSUM banks: sp 2x2 + ctx 2 + ps 1 + tp 1 = 8
            tc.tile_pool(name="spp", bufs=2, space="PSUM") as spp,
            tc.tile_pool(name="pctx", bufs=2, space="PSUM") as pctx,
            tc.tile_pool(name="psmall", bufs=1, space="PSUM") as psmall,
        ):
            # ---------- weights first (first c-chunks split out so the
            # very first projection matmuls can start early) ----
            wk_sb = wpool.tile([P, EC, DPC], BF16, name="wk_sb")
            nc.sync.dma_start(
                wk_sb[:, 0:2], WkT.ap()[:, : 2 * DPC].rearrange("p (c d) -> p c d", c=2)
            )
            # identity via DMA (host-prepared): ready ~9us, feeds the PE
            # warm-up chain — gpsimd make_identity would land far too late
            ident_bf = consts.tile([P, P], BF16, name="ident_bf")
            nc.scalar.dma_start(ident_bf, idn.ap())
            wq_sb = wpool.tile([P, EC, DPC], BF16, name="wq_sb")
            nc.scalar.dma_start(
                wq_sb[:, 0:2], WqT.ap()[:, : 2 * DPC].rearrange("p (c d) -> p c d", c=2)
            )
            nc.sync.dma_start(
                wk_sb[:, 2:], WkT.ap()[:, 2 * DPC :].rearrange("p (c d) -> p c d", c=EC - 2)
            )
            nc.scalar.dma_start(
                wq_sb[:, 2:], WqT.ap()[:, 2 * DPC :].rearrange("p (c d) -> p c d", c=EC - 2)
            )

            # ---------- source DMAs (chunked along E-contraction) --------
            src_chunks = {}

            def dma_src(which, b, r, lo, hi, eng):
                dram = qTt if which == "q" else eTt
                tag = f"{which}{hi - lo}"
                tl = (qsrc if which == "q" else esrc).tile(
                    [P, hi - lo, NFREE], BF16, tag=tag, bufs=(4 if hi - lo == 2 else 8),
                    name=f"{which}{b}{r}c{lo}",
                )
                eng.dma_start(
                    tl,
                    dram.ap()[:, b * RC_B + r, lo * NFREE : hi * NFREE].rearrange(
                        "p (c n) -> p c n", c=hi - lo
                    ),
                )
                src_chunks.setdefault((which, b, r), []).append((tl, lo, hi))

            def sl(which, b, r, c):
                for tl, lo, hi in src_chunks[(which, b, r)]:
                    if lo <= c < hi:
                        return tl[:, c - lo]
                raise KeyError((which, b, r, c))

            # startup-critical quarters: e00 on sync+scalar, q00 on
            # gpsimd+scalar (3 DMA-capable rings: sync, scalar, gpsimd)
            # ~0.95MB per ring so kproj+qproj inputs all land ~16-17.5us
            dma_src("e", 0, 0, 0, 2, nc.sync)
            dma_src("e", 0, 0, 4, 6, nc.scalar)
            dma_src("q", 0, 0, 0, 2, nc.gpsimd)
            dma_src("q", 0, 0, 4, 6, nc.sync)
            dma_src("e", 0, 0, 6, 8, nc.scalar)
            dma_src("q", 0, 0, 2, 4, nc.gpsimd)
            dma_src("q", 0, 0, 6, 8, nc.sync)
            dma_src("e", 0, 0, 2, 4, nc.gpsimd)
            # wv + biases on gpsimd (needed by vproj from G0ish — must be
            # queued ahead of the bulky q01-03 halves)
            wv_sb = wpool.tile([P, EC, DPC], BF16, name="wv_sb")
            nc.gpsimd.dma_start(wv_sb, WvT.ap().rearrange("p (c d) -> p c d", c=EC))
            bq_sb = wpool.tile([1, DPC], BF16, name="bq_sb")
            nc.gpsimd.dma_start(bq_sb, bqs.ap()[None, :])
            bk_sb = wpool.tile([1, DPC], BF16, name="bk_sb")
            nc.gpsimd.dma_start(bk_sb, bkp.ap()[None, :])
            bv_sb = wpool.tile([1, DPC], BF16, name="bv_sb")
            nc.gpsimd.dma_start(bv_sb, bvp.ap()[None, :])
            # b0 e-halves spread over all three rings, ordered by need time
            # (e01 by ~G2, e02 by ~G6, e03 by ~G10)
            dma_src("e", 0, 1, 0, 4, nc.sync)
            dma_src("e", 0, 1, 4, 8, nc.scalar)
            dma_src("e", 0, 2, 0, 4, nc.gpsimd)
            dma_src("e", 0, 2, 4, 8, nc.scalar)
            dma_src("e", 0, 3, 0, 4, nc.sync)
            dma_src("e", 0, 3, 4, 8, nc.gpsimd)
            for r in (1, 2, 3):
                dma_src("q", 0, r, 0, 4, nc.gpsimd)
                dma_src("q", 0, r, 4, 8, nc.gpsimd)

            # ---------- constants ----------
            ones_row = consts.tile([1, NFREE], BF16)
            nc.vector.memset(ones_row, 1.0)
            # warm the PE HAM (flip to 2.4 GHz) during the initial DMA wait.
            # K=128 dense matmuls on the identity (K=1 or tiny-N chains keep
            # the array duty too low to trip the HAM busy window).
            wmp = psmall.tile([P, NFREE], F32, tag="ps", name="wmp")

            def warm_mm(n):
                for _ in range(n):
                    nc.tensor.matmul(
                        wmp[:, 0:P], lhsT=ident_bf, rhs=ident_bf,
                        start=True, stop=True,
                    )

            warm_mm(24)
            # warm the ACT exp table (after scalar's DMA issues)
            warm = consts.tile([1, 1], BF16)
            nc.scalar.activation(warm, ones_row[:, 0:1], AF.Exp)

            # ---------- residents (per batch) ----------
            qt_sb = []
            kt_sb = []
            v_sb = []
            for b in range(B):
                qt = resid.tile([P, S], BF16, name=f"qt{b}")
                kt = resid.tile([P, S], BF16, name=f"kt{b}")
                vv = resid.tile([P, KC, HPC, HD + 1], BF16, name=f"v{b}")
                nc.vector.memset(vv[:, :, :, HD : HD + 1], 1.0)
                qt_sb.append(qt)
                kt_sb.append(kt)
                v_sb.append(vv)

            # ---------- projections ----------
            def qk_proj(b, r, which, pool_, tag):
                if which == "q":
                    w_t, b_t, dst = wq_sb, bq_sb, qt_sb[b]
                else:
                    w_t, b_t, dst = wk_sb, bk_sb, kt_sb[b]
                pp = pool_.tile([P, NFREE], F32, tag=tag, name=f"pp{which}{b}_{r}")
                for c in range(EC):
                    nc.tensor.matmul(
                        pp,
                        lhsT=w_t[:, c],
                        rhs=sl(which, b, r, c),
                        start=(c == 0),
                        stop=(not with_bias and c == EC - 1),
                    )
                if with_bias:
                    nc.tensor.matmul(pp, lhsT=b_t, rhs=ones_row, start=False, stop=True)
                nc.vector.tensor_copy(dst[:, r * NFREE : (r + 1) * NFREE], pp)

            def v_proj(b, r, half):
                for sub in (2 * half, 2 * half + 1):
                    kc = r * TPB + sub
                    pv = psmall.tile([P, DPC], F32, tag="ps", name=f"pv{b}_{kc}")
                    for c in range(EC):
                        nc.tensor.matmul(
                            pv,
                            lhsT=sl("e", b, r, c)[:, sub * P : (sub + 1) * P],
                            rhs=wv_sb[:, c],
                            start=(c == 0),
                            stop=(not with_bias and c == EC - 1),
                        )
                    if with_bias:
                        nc.tensor.matmul(
                            pv, lhsT=ones_row[:, :P], rhs=bv_sb, start=False, stop=True
                        )
                    for h in range(HPC):
                        nc.vector.tensor_copy(
                            v_sb[b][:, kc, h, 0:HD], pv[:, h * HD : (h + 1) * HD]
                        )

            # ---------- per-block ctx/norm/out tasks ----------
            pr_holder = {}
            T = _Task

            def make_ctx_tasks(bi):
                b, qc = ORDER[bi]
                base = NG * bi
                ctxps = {}
                ctxTs = {}
                ot = otp.tile([P, TPB, DPC], F32, tag="ot", name=f"ot{bi}")

                def ctx_chunk(h, j):
                    def run():
                        pr = pr_holder[bi]
                        if j == 0:
                            ctxps[h] = pctx.tile(
                                [HD + 1, NFREE], F32, tag="ctx", name=f"ctx{bi}_{h}"
                            )
                        cp = ctxps[h]
                        for kc in (2 * j, 2 * j + 1):
                            nc.tensor.matmul(
                                cp,
                                lhsT=v_sb[b][:, kc, h, :],
                                rhs=pr[:, 2 * kc + h, :],
                                start=(kc == 0),
                                stop=(kc == KC - 1),
                            )
                    return run

                def ctx_kc(h, kc):
                    def run():
                        pr = pr_holder[bi]
                        nc.tensor.matmul(
                            ctxps[h],
                            lhsT=v_sb[b][:, kc, h, :],
                            rhs=pr[:, 2 * kc + h, :],
                            start=False,
                            stop=(kc == KC - 1),
                        )
                    return run

                def drain(h):
                    def run():
                        ctxTs[h] = misc.tile(
                            [HD + 1, NFREE], BF16, tag="ctxT", name=f"ctxT{bi}_{h}"
                        )
                        nc.vector.tensor_copy(ctxTs[h], ctxps[h])
                    return run

                def norm(h, tlo=0, thi=TPB, ptag="tp"):
                    def run():
                        nt = thi - tlo
                        tp = psmall.tile(
                            [P, nt, 80], BF16, tag=ptag, name=f"tp{bi}_{h}_{tlo}"
                        )
                        for t in range(tlo, thi):
                            nc.tensor.transpose(
                                tp[:, t - tlo, 0 : HD + 1],
                                ctxTs[h][:, t * P : (t + 1) * P],
                                ident_bf[: HD + 1, : HD + 1],
                            )
                        rcp = misc.tile(
                            [P, nt, 1], F32, tag="rcp", bufs=4, name=f"rcp{bi}_{h}_{tlo}"
                        )
                        nc.vector.reciprocal(rcp, tp[:, :, HD : HD + 1])
                        nc.vector.tensor_mul(
                            ot[:, tlo:thi, h * HD : (h + 1) * HD],
                            tp[:, :, 0:HD],
                            rcp.broadcast_to([P, nt, HD]),
                        )
                    return run

                def dma_out(tlo, thi):
                    def run():
                        row0 = b * S + qc * NFREE + tlo * P
                        nc.sync.dma_start(
                            out.ap()[row0 : row0 + (thi - tlo) * P, :].rearrange(
                                "(t p) d -> p t d", p=P
                            ),
                            ot[:, tlo:thi, :],
                        )
                    return run

                tasks = []
                # b0's e-tiles (-> V kc8-15) only land ~G12-15, so block 0's
                # late ctx chunks shift into block 1's window; block 1 then
                # waits for block 0's pctx drains, block 2 half-recovers and
                # block 3+ runs the steady-state template.
                if bi == 0:
                    cg = [4, 6, 8, 10, 20, 22, 24, 26]
                    dr, nr, dm = 28, 30, 32
                elif bi == 1:
                    cg = [30, 31, 32, 33, 34, 35, 36, 37]
                    dr, nr, dm = 38, 40, 42
                elif bi == 2:
                    # j7 reads group 47's slots -> gate 48 (after exp(47))
                    cg = [40, 41, 42, 43, 44, 45, 46, 48]
                    dr, nr, dm = 49, 50, 52
                elif bi >= 4:
                    # the back half is ACT-bound with PE slack: shift ctx
                    # late to unload the front, tapering so each block's
                    # pctx drain still precedes the next block's alloc
                    s = 8 - bi  # 4,3,2,1 for blocks 4-7
                    cg = [base + 2 * j + 2 + s for j in range(8)]
                    dr, nr, dm = base + 17 + s, base + 18 + s, base + 20 + s
                else:
                    cg = [base + 2 * j + 2 for j in range(8)]
                    dr, nr, dm = base + 17, base + 18, base + 20
                for j in range(8):
                    if bi == NB - 1 and j == 7:
                        # split the final chunk so only one 1-kc matmul per
                        # head remains after the last exp group
                        for h in range(HPC):
                            tasks.append(T(base + 15, base + 19, 215, ctx_kc(h, 14)))
                        for h in range(HPC):
                            tasks.append(T(base + 16, base + 20, 215, ctx_kc(h, 15)))
                        continue
                    for h in range(HPC):
                        tasks.append(T(cg[j] + h, cg[j] + h + 4, 430, ctx_chunk(h, j)))
                if bi < NB - 1:
                    for h in range(HPC):
                        tasks.append(T(dr + h, dr + h + 4, 100, drain(h)))
                        tasks.append(T(nr + h, nr + h + 5, 600, norm(h)))
                    tasks.append(T(dm, dm + 6, 0, dma_out(0, TPB)))
                else:
                    # fine-grained tail: 2-t norm chunks and split out-DMAs
                    # so the post-exp critical chain is as short as possible
                    tasks.append(T(base + 17, base + 23, 100, drain(0)))
                    tasks.append(T(base + 18, base + 24, 300, norm(0, 0, 2, "tp")))
                    tasks.append(T(base + 18, base + 25, 100, drain(1)))
                    tasks.append(T(base + 19, base + 25, 300, norm(0, 2, TPB, "ps")))
                    tasks.append(T(base + 19, base + 26, 300, norm(1, 0, 2, "tp")))
                    tasks.append(T(base + 20, base + 26, 0, dma_out(0, 2)))
                    tasks.append(T(base + 20, base + 27, 300, norm(1, 2, TPB, "ps")))
                    tasks.append(T(base + 21, base + 27, 0, dma_out(2, TPB)))
                return tasks

            # ---------- global filler task list ----------
            fillers = []

            def qk_task(b, r, which, gate, dl):
                fillers.append(
                    T(gate, dl, 1800, lambda b=b, r=r, w=which: qk_proj(b, r, w, psmall, "ps"))
                )

            def v_task(b, r, half, gate, dl):
                fillers.append(
                    T(gate, dl, 1040, lambda b=b, r=r, hf=half: v_proj(b, r, hf))
                )

            # b0 prep: e r1-3 proj feed score kc 4..15 (group == kc); gates
            # match expected DMA arrival so pops don't block the PE queue
            qk_task(0, 1, "e", 0, 4)
            qk_task(0, 2, "e", 4, 8)
            qk_task(0, 3, "e", 8, 12)
            v_task(0, 0, 0, 0, 4)
            v_task(0, 0, 1, 1, 6)
            v_task(0, 1, 0, 4, 8)
            v_task(0, 1, 1, 5, 10)
            v_task(0, 2, 0, 8, 20)
            v_task(0, 2, 1, 9, 22)
            v_task(0, 3, 0, 12, 24)
            v_task(0, 3, 1, 13, 26)
            qk_task(0, 1, "q", 6, 16)
            qk_task(0, 2, "q", 18, 32)
            qk_task(0, 3, "q", 30, 48)

            # b1 source DMA issues (esrc/qsrc rings are deep enough that
            # slot reuse only needs b0-r-readers of the same slot emitted)
            def dma_b1(which, r, eng):
                def run():
                    dma_src(which, 1, r, 0, 4, eng)
                    dma_src(which, 1, r, 4, 8, eng)
                return run

            for r in range(RC_B):
                fillers.append(T(12 + r, 30 + 4 * r, 0, dma_b1("e", r, nc.sync)))
            # q-ring slot reuse: q1r2/q1r3 land on q02/q03 slots, whose
            # readers qproj(0,2)/(0,3) are emitted by G30/G46 (deadlines).
            for r, gate in enumerate((17, 18, 31, 47)):
                fillers.append(T(gate, 48 + 4 * r, 0, dma_b1("q", r, nc.gpsimd)))

            # b1 prep — gated as early as the b1 DMAs plausibly land, with
            # deadlines a few groups before the scores that need them so the
            # work spreads over blocks 2-3 instead of bunching at G62-79
            qk_task(1, 0, "e", 28, 54)
            qk_task(1, 1, "e", 32, 56)
            qk_task(1, 2, "e", 36, 58)
            qk_task(1, 3, "e", 40, 60)
            qk_task(1, 0, "q", 42, 60)
            # vproj(1) is only needed from G70 (shifted ctx(4)) — park it in
            # the ACT-bound back half instead of the oversubscribed front
            for r in range(RC_B):
                for half in (0, 1):
                    j = 2 * r + half
                    v_task(1, r, half, 56 + 2 * j, 70 + 2 * j)
            qk_task(1, 1, "q", 64, 78)
            qk_task(1, 2, "q", 72, 94)
            qk_task(1, 3, "q", 88, 110)

            for bi in range(NB):
                fillers += make_ctx_tasks(bi)

            fillers.sort(key=lambda t: (t.gate, t.deadline))
            total_cost = sum(t.cost for t in fillers)

            # ---------- startup projections (r0 of b0) ----------
            # warm matmuls interleaved between the DMA-paced chunks keep the
            # PE array duty high so the HAM reaches 2.4 GHz by ~12us
            pp_e0 = spp.tile([P, NFREE], F32, tag="sp", name="pp_e00")
            pp_q0 = spp.tile([P, NFREE], F32, tag="sp", name="pp_q00")
            for c in range(EC):
                nc.tensor.matmul(
                    pp_e0, lhsT=wk_sb[:, c], rhs=sl("e", 0, 0, c),
                    start=(c == 0), stop=(not with_bias and c == EC - 1),
                )
                warm_mm(2)
            if with_bias:
                nc.tensor.matmul(pp_e0, lhsT=bk_sb, rhs=ones_row, start=False, stop=True)
            nc.vector.tensor_copy(kt_sb[0][:, 0:NFREE], pp_e0)
            for c in range(EC):
                nc.tensor.matmul(
                    pp_q0, lhsT=wq_sb[:, c], rhs=sl("q", 0, 0, c),
                    start=(c == 0), stop=(not with_bias and c == EC - 1),
                )
                warm_mm(1)
            if with_bias:
                nc.tensor.matmul(pp_q0, lhsT=bq_sb, rhs=ones_row, start=False, stop=True)
            nc.vector.tensor_copy(qt_sb[0][:, 0:NFREE], pp_q0)

            # ---------- flat attention stream ----------
            def scores_for(X):
                bi, g = divmod(X, NG)
                b, qc = ORDER[bi]
                col0 = qc * NFREE
                sp = spp.tile([P, HPC, NFREE], F32, tag="sp", name=f"sp{X}")
                for h in range(HPC):
                    d0 = h * HD
                    nc.tensor.matmul(
                        sp[:, h, :],
                        lhsT=kt_sb[b][d0 : d0 + HD, g * P : (g + 1) * P],
                        rhs=qt_sb[b][d0 : d0 + HD, col0 : col0 + NFREE],
                        start=True,
                        stop=True,
                    )
                return sp

            sps = {0: scores_for(0), 1: scores_for(1)}
            done_cost = 0.0
            for G in range(NGT):
                bi, g = divmod(G, NG)
                if g == 0:
                    pr_holder[bi] = prp.tile(
                        [P, NSLOT, NFREE], BF16, tag="pr", name=f"pr{bi}"
                    )
                # forced pops: anything that must precede scores(G+2)
                i = 0
                while i < len(fillers):
                    if fillers[i].deadline <= G + 2:
                        t = fillers.pop(i)
                        t.fn()
                        done_cost += t.cost
                    else:
                        i += 1
                # budget pops: gate-ready work, rate-limited so a burst of
                # ready tasks cannot starve the exp stream
                want = total_cost * (G + 1) / NGT
                while fillers and fillers[0].gate <= G and done_cost < want:
                    t = fillers.pop(0)
                    t.fn()
                    done_cost += t.cost
                if G + 2 < NGT:
                    sps[G + 2] = scores_for(G + 2)
                pr = pr_holder[bi]
                nc.scalar.activation(
                    pr[:, HPC * g : HPC * g + HPC, :], sps.pop(G), AF.Exp
                )
            while fillers:
                fillers.pop(0).fn()

    nc.finalize()
    return nc


def _get_nc(with_bias: bool = True) -> bass.Bass:
    if with_bias not in _CACHED_NC:
        _CACHED_NC[with_bias] = _build_nc(with_bias)
    return _CACHED_NC[with_bias]


def kernel(embed, q, Wk, bk, Wq, bq, Wv, bv, trace=False):
    global LAST_RESULTS
    bf = ml_dtypes.bfloat16
    embed = np.asarray(embed, dtype=np.float32)
    q = np.asarray(q, dtype=np.float32)
    Wk = np.asarray(Wk, dtype=np.float32)
    Wq = np.asarray(Wq, dtype=np.float32)
    Wv = np.asarray(Wv, dtype=np.float32)
    bk = np.asarray(bk, dtype=np.float32)
    bq = np.asarray(bq, dtype=np.float32)
    bv = np.asarray(bv, dtype=np.float32)

    qTt = _tile_inputs(q.reshape(ROWS, E))
    eTt = _tile_inputs(embed.reshape(ROWS, E))

    in_maps = []
    for c in range(NCORES):
        sl = slice(c * DPC, (c + 1) * DPC)
        in_maps.append(
            {
                "qTt": qTt,
                "eTt": eTt,
                # scores scale folded into Wq/bq (exact: *2^-3)
                "WqT": _tile_w(np.ascontiguousarray((Wq[sl] * SCALE).T).astype(bf)),
                "WkT": _tile_w(np.ascontiguousarray(Wk[sl].T).astype(bf)),
                "WvT": _tile_w(np.ascontiguousarray(Wv[sl].T).astype(bf)),
                "bqs": (bq[sl] * SCALE).astype(bf),
                "bkp": bk[sl].astype(bf),
                "bvp": bv[sl].astype(bf),
                "idn": np.eye(P, dtype=np.float32).astype(bf),
            }
        )

    with_bias = bool(bq.any() or bk.any() or bv.any())
    nc = _get_nc(with_bias)
    res = run_bass_kernel_spmd(nc, in_maps, list(range(NCORES)), trace=trace)
    LAST_RESULTS = res

    full = np.empty((ROWS, E), dtype=np.float32)
    for c in range(NCORES):
        full[:, c * DPC : (c + 1) * DPC] = res.results[c]["out"]
    return full.reshape(B, S, E)


# revision 44
# speedup vs baseline: 1.0820x; 1.0172x over previous
"""Cross multi-head attention TRN2 kernel (8-core SPMD, head-sharded), v2.

Strategy (tensor parallel over heads, zero communication):
  - 16 heads / 8 cores -> 2 heads per core. Core c computes output columns
    [128*c, 128*(c+1)) of the [4096, 1024] output; host concatenates.
  - Host pre-transposes and PRE-TILES q/embed into [tile, P, chunks] bf16
    layouts so every input DMA is contiguous per partition.
  - Scores are computed transposed (S^T[k, q] = K.Q^T, scale folded into Wq).
    The two heads per core live on partition halves 0-63 / 64-127, so their
    K=64 score matmuls row-tile and overlap ~2x on the PE.
  - The exp stream is 128 uniform groups of 2 score slots (one kc, both
    heads) x [P, 2, 512] fp32 PSUM, double buffered (4 banks); ACT is the
    near-critical engine (~130us) and the PE (~140us) paces the kernel.
  - All other PE work (attn@V 2-kc chunks, projections, V-proj, ctx
    transposes) is a task list with (gate, deadline) bounds popped between
    exp groups; deadlines are tight so work never bursts at block edges.
  - Softmax denominator via a ones-column appended to V (attn@V also
    produces row-sums); ctx'^T is PE-transposed back to [q, d] (bf16),
    normalized per-partition (DVE reciprocal+mul), DMA'd out per block.
"""

import numpy as np
import ml_dtypes

import concourse.bass as bass
import concourse.bacc as bacc
import concourse.mybir as mybir
import concourse.tile as tile
from concourse.bass_utils import run_bass_kernel_spmd

# ---- problem dims (hardcoded; kernel.py must be self-contained) ----
B, S, E = 2, 2048, 1024
NHEAD, HD = 16, 64
NCORES = 8
HPC = NHEAD // NCORES          # heads per core = 2
DPC = HPC * HD                 # projection out-dims per core = 128
ROWS = B * S                   # 4096
P = 128                        # SBUF partitions
NFREE = 512                    # matmul moving free dim (one PSUM bank fp32)
EC = E // P                    # 8 contraction chunks
KC = S // P                    # 16 key chunks per batch
QC = S // NFREE                # 4 query chunks per batch
RC_B = S // NFREE              # 4 projection row-chunks per batch
TPB = NFREE // P               # 4 transpose chunks per block
NSLOT = HPC * KC               # 32 score slots per (b,qc) block
NG = KC                        # 16 exp groups per block (2 slots each)
NB = B * QC                    # 8 blocks
NGT = NB * NG                  # 128 groups total
SCALE = 1.0 / np.sqrt(HD)      # 0.125, folded into Wq/bq on host

F32 = mybir.dt.float32
BF16 = mybir.dt.bfloat16
AF = mybir.ActivationFunctionType

_CACHED_NC = {}
LAST_RESULTS = None            # test.py reads exec_time_ns / profile from here

ORDER = [(0, 0), (0, 1), (0, 2), (0, 3), (1, 0), (1, 1), (1, 2), (1, 3)]


def _tile_inputs(mat_rows_e: np.ndarray) -> np.ndarray:
    """[ROWS, E] f32 -> pre-tiled bf16 [P, B*RC_B, EC*NFREE]: slice
    [:, b*RC_B+r, :] is one projection row-chunk, 8KB contiguous per
    partition, laid out [c, n] with E-index = c*128 + p, col = b*S +
    r*512 + n."""
    t = np.ascontiguousarray(mat_rows_e.T).astype(ml_dtypes.bfloat16)
    a = t.reshape(EC, P, B, RC_B, NFREE)            # [c, p, b, r, n]
    a = a.transpose(1, 2, 3, 0, 4)                  # [p, b, r, c, n]
    return np.ascontiguousarray(a.reshape(P, B * RC_B, EC * NFREE))


def _tile_w(wT: np.ndarray) -> np.ndarray:
    """[E, DPC] bf16 weight -> [P, EC*DPC]: contiguous per partition."""
    a = wT.reshape(EC, P, DPC).transpose(1, 0, 2)
    return np.ascontiguousarray(a.reshape(P, EC * DPC))


class _Task:
    """Filler work item: gate = earliest group index after whose exp it may
    be emitted; deadline = group whose SCORES it must precede (forced-pop
    at G >= deadline-2); cost = PE-ns estimate for pacing."""

    __slots__ = ("gate", "deadline", "cost", "fn")

    def __init__(self, gate, deadline, cost, fn):
        self.gate = gate
        self.deadline = deadline
        self.cost = cost
        self.fn = fn


def _build_nc(with_bias: bool) -> bass.Bass:
    nc = bacc.Bacc(
        "TRN2",
        target_bir_lowering=False,
        debug=False,
        num_devices=NCORES,
    )

    qTt = nc.declare_dram_parameter("qTt", [P, B * RC_B, EC * NFREE], BF16, isOutput=False)
    eTt = nc.declare_dram_parameter("eTt", [P, B * RC_B, EC * NFREE], BF16, isOutput=False)
    WqT = nc.declare_dram_parameter("WqT", [P, EC * DPC], BF16, isOutput=False)
    WkT = nc.declare_dram_parameter("WkT", [P, EC * DPC], BF16, isOutput=False)
    WvT = nc.declare_dram_parameter("WvT", [P, EC * DPC], BF16, isOutput=False)
    bqs = nc.declare_dram_parameter("bqs", [DPC], BF16, isOutput=False)
    bkp = nc.declare_dram_parameter("bkp", [DPC], BF16, isOutput=False)
    bvp = nc.declare_dram_parameter("bvp", [DPC], BF16, isOutput=False)
    idn = nc.declare_dram_parameter("idn", [P, P], BF16, isOutput=False)
    out = nc.declare_dram_parameter("out", [ROWS, DPC], F32, isOutput=True)

    with tile.TileContext(nc) as tc:
        with (
            tc.tile_pool(name="consts", bufs=1) as consts,
            tc.tile_pool(name="wpool", bufs=1) as wpool,
            tc.tile_pool(name="resid", bufs=1) as resid,
            tc.tile_pool(name="esrc", bufs=1) as esrc,
            tc.tile_pool(name="qsrc", bufs=1) as qsrc,
            tc.tile_pool(name="prp", bufs=2) as prp,
            tc.tile_pool(name="misc", bufs=2) as misc,
            tc.tile_pool(name="otp", bufs=4) as otp,
            # PSUM banks: sp 2x2 + ctx 2 + ps 1 + tp 1 = 8
            tc.tile_pool(name="spp", bufs=2, space="PSUM") as spp,
            tc.tile_pool(name="pctx", bufs=2, space="PSUM") as pctx,
            tc.tile_pool(name="psmall", bufs=1, space="PSUM") as psmall,
        ):
            # ---------- weights first (first c-chunks split out so the
            # very first projection matmuls can start early) ----
            wk_sb = wpool.tile([P, EC, DPC], BF16, name="wk_sb")
            nc.sync.dma_start(
                wk_sb[:, 0:2], WkT.ap()[:, : 2 * DPC].rearrange("p (c d) -> p c d", c=2)
            )
            # identity via DMA (host-prepared): ready ~9us, feeds the PE
            # warm-up chain — gpsimd make_identity would land far too late
            ident_bf = consts.tile([P, P], BF16, name="ident_bf")
            nc.scalar.dma_start(ident_bf, idn.ap())
            wq_sb = wpool.tile([P, EC, DPC], BF16, name="wq_sb")
            nc.scalar.dma_start(
                wq_sb[:, 0:2], WqT.ap()[:, : 2 * DPC].rearrange("p (c d) -> p c d", c=2)
            )
            nc.sync.dma_start(
                wk_sb[:, 2:], WkT.ap()[:, 2 * DPC :].rearrange("p (c d) -> p c d", c=EC - 2)
            )
            nc.scalar.dma_start(
                wq_sb[:, 2:], WqT.ap()[:, 2 * DPC :].rearrange("p (c d) -> p c d", c=EC - 2)
            )

            # ---------- source DMAs (chunked along E-contraction) --------
            src_chunks = {}

            def dma_src(which, b, r, lo, hi, eng):
                dram = qTt if which == "q" else eTt
                tag = f"{which}{hi - lo}"
                tl = (qsrc if which == "q" else esrc).tile(
                    [P, hi - lo, NFREE], BF16, tag=tag, bufs=(4 if hi - lo == 2 else 8),
                    name=f"{which}{b}{r}c{lo}",
                )
                eng.dma_start(
                    tl,
                    dram.ap()[:, b * RC_B + r, lo * NFREE : hi * NFREE].rearrange(
                        "p (c n) -> p c n", c=hi - lo
                    ),
                )
                src_chunks.setdefault((which, b, r), []).append((tl, lo, hi))

            def sl(which, b, r, c):
                for tl, lo, hi in src_chunks[(which, b, r)]:
                    if lo <= c < hi:
                        return tl[:, c - lo]
                raise KeyError((which, b, r, c))

            # startup-critical quarters: e00 on sync+scalar, q00 on
            # gpsimd+scalar (3 DMA-capable rings: sync, scalar, gpsimd)
            # ~0.95MB per ring so kproj+qproj inputs all land ~16-17.5us
            dma_src("e", 0, 0, 0, 2, nc.sync)
            dma_src("e", 0, 0, 4, 6, nc.scalar)
            dma_src("q", 0, 0, 0, 2, nc.gpsimd)
            dma_src("q", 0, 0, 4, 6, nc.sync)
            dma_src("e", 0, 0, 6, 8, nc.scalar)
            dma_src("q", 0, 0, 2, 4, nc.gpsimd)
            dma_src("q", 0, 0, 6, 8, nc.sync)
            dma_src("e", 0, 0, 2, 4, nc.gpsimd)
            # wv + biases on gpsimd (needed by vproj from G0ish — must be
            # queued ahead of the bulky q01-03 halves)
            wv_sb = wpool.tile([P, EC, DPC], BF16, name="wv_sb")
            nc.gpsimd.dma_start(wv_sb, WvT.ap().rearrange("p (c d) -> p c d", c=EC))
            bq_sb = wpool.tile([1, DPC], BF16, name="bq_sb")
            nc.gpsimd.dma_start(bq_sb, bqs.ap()[None, :])
            bk_sb = wpool.tile([1, DPC], BF16, name="bk_sb")
            nc.gpsimd.dma_start(bk_sb, bkp.ap()[None, :])
            bv_sb = wpool.tile([1, DPC], BF16, name="bv_sb")
            nc.gpsimd.dma_start(bv_sb, bvp.ap()[None, :])
            # b0 e-halves spread over all three rings, ordered by need time
            # (e01 by ~G2, e02 by ~G6, e03 by ~G10)
            dma_src("e", 0, 1, 0, 4, nc.sync)
            dma_src("e", 0, 1, 4, 8, nc.scalar)
            dma_src("e", 0, 2, 0, 4, nc.gpsimd)
            dma_src("e", 0, 2, 4, 8, nc.scalar)
            dma_src("e", 0, 3, 0, 4, nc.sync)
            dma_src("e", 0, 3, 4, 8, nc.gpsimd)
            for r in (1, 2, 3):
                dma_src("q", 0, r, 0, 4, nc.gpsimd)
                dma_src("q", 0, r, 4, 8, nc.gpsimd)

            # ---------- constants ----------
            ones_row = consts.tile([1, NFREE], BF16)
            nc.vector.memset(ones_row, 1.0)
            # warm the PE HAM (flip to 2.4 GHz) during the initial DMA wait.
            # The HAM flips only after a FULL free-running 3.4us window of
            # sustained array streaming, so the chain must span >=7us of
            # dense K=128 matmuls. wk's first chunk lands ~9.7us (first on
            # the sync ring) — warm on it so the chain needs no extra input.
            wmp = psmall.tile([P, NFREE], F32, tag="ps", name="wmp")
            for _ in range(33):
                nc.tensor.matmul(
                    wmp[:, 0:256], lhsT=wk_sb[:, 0], rhs=wk_sb[:, 0:2],
                    start=True, stop=True,
                )
            # warm the ACT exp table (after scalar's DMA issues)
            warm = consts.tile([1, 1], BF16)
            nc.scalar.activation(warm, ones_row[:, 0:1], AF.Exp)

            # ---------- residents (per batch) ----------
            qt_sb = []
            kt_sb = []
            v_sb = []
            for b in range(B):
                qt = resid.tile([P, S], BF16, name=f"qt{b}")
                kt = resid.tile([P, S], BF16, name=f"kt{b}")
                vv = resid.tile([P, KC, HPC, HD + 1], BF16, name=f"v{b}")
                nc.vector.memset(vv[:, :, :, HD : HD + 1], 1.0)
                qt_sb.append(qt)
                kt_sb.append(kt)
                v_sb.append(vv)

            # ---------- projections ----------
            def qk_proj(b, r, which, pool_, tag):
                if which == "q":
                    w_t, b_t, dst = wq_sb, bq_sb, qt_sb[b]
                else:
                    w_t, b_t, dst = wk_sb, bk_sb, kt_sb[b]
                pp = pool_.tile([P, NFREE], F32, tag=tag, name=f"pp{which}{b}_{r}")
                for c in range(EC):
                    nc.tensor.matmul(
                        pp,
                        lhsT=w_t[:, c],
                        rhs=sl(which, b, r, c),
                        start=(c == 0),
                        stop=(not with_bias and c == EC - 1),
                    )
                if with_bias:
                    nc.tensor.matmul(pp, lhsT=b_t, rhs=ones_row, start=False, stop=True)
                nc.vector.tensor_copy(dst[:, r * NFREE : (r + 1) * NFREE], pp)

            def v_proj(b, r, half):
                for sub in (2 * half, 2 * half + 1):
                    kc = r * TPB + sub
                    pv = psmall.tile([P, DPC], F32, tag="ps", name=f"pv{b}_{kc}")
                    for c in range(EC):
                        nc.tensor.matmul(
                            pv,
                            lhsT=sl("e", b, r, c)[:, sub * P : (sub + 1) * P],
                            rhs=wv_sb[:, c],
                            start=(c == 0),
                            stop=(not with_bias and c == EC - 1),
                        )
                    if with_bias:
                        nc.tensor.matmul(
                            pv, lhsT=ones_row[:, :P], rhs=bv_sb, start=False, stop=True
                        )
                    for h in range(HPC):
                        nc.vector.tensor_copy(
                            v_sb[b][:, kc, h, 0:HD], pv[:, h * HD : (h + 1) * HD]
                        )

            # ---------- per-block ctx/norm/out tasks ----------
            pr_holder = {}
            T = _Task

            def make_ctx_tasks(bi):
                b, qc = ORDER[bi]
                base = NG * bi
                ctxps = {}
                ctxTs = {}
                ot = otp.tile([P, TPB, DPC], F32, tag="ot", name=f"ot{bi}")

                def ctx_chunk(h, j):
                    def run():
                        pr = pr_holder[bi]
                        if j == 0:
                            ctxps[h] = pctx.tile(
                                [HD + 1, NFREE], F32, tag="ctx", name=f"ctx{bi}_{h}"
                            )
                        cp = ctxps[h]
                        for kc in (2 * j, 2 * j + 1):
                            nc.tensor.matmul(
                                cp,
                                lhsT=v_sb[b][:, kc, h, :],
                                rhs=pr[:, 2 * kc + h, :],
                                start=(kc == 0),
                                stop=(kc == KC - 1),
                            )
                    return run

                def ctx_kc(h, kc):
                    def run():
                        pr = pr_holder[bi]
                        nc.tensor.matmul(
                            ctxps[h],
                            lhsT=v_sb[b][:, kc, h, :],
                            rhs=pr[:, 2 * kc + h, :],
                            start=False,
                            stop=(kc == KC - 1),
                        )
                    return run

                def drain(h):
                    def run():
                        ctxTs[h] = misc.tile(
                            [HD + 1, NFREE], BF16, tag="ctxT", name=f"ctxT{bi}_{h}"
                        )
                        nc.vector.tensor_copy(ctxTs[h], ctxps[h])
                    return run

                def norm(h, tlo=0, thi=TPB, ptag="tp"):
                    def run():
                        nt = thi - tlo
                        tp = psmall.tile(
                            [P, nt, 80], BF16, tag=ptag, name=f"tp{bi}_{h}_{tlo}"
                        )
                        for t in range(tlo, thi):
                            nc.tensor.transpose(
                                tp[:, t - tlo, 0 : HD + 1],
                                ctxTs[h][:, t * P : (t + 1) * P],
                                ident_bf[: HD + 1, : HD + 1],
                            )
                        rcp = misc.tile(
                            [P, nt, 1], F32, tag="rcp", bufs=4, name=f"rcp{bi}_{h}_{tlo}"
                        )
                        nc.vector.reciprocal(rcp, tp[:, :, HD : HD + 1])
                        nc.vector.tensor_mul(
                            ot[:, tlo:thi, h * HD : (h + 1) * HD],
                            tp[:, :, 0:HD],
                            rcp.broadcast_to([P, nt, HD]),
                        )
                    return run

                def dma_out(tlo, thi):
                    def run():
                        row0 = b * S + qc * NFREE + tlo * P
                        nc.sync.dma_start(
                            out.ap()[row0 : row0 + (thi - tlo) * P, :].rearrange(
                                "(t p) d -> p t d", p=P
                            ),
                            ot[:, tlo:thi, :],
                        )
                    return run

                tasks = []
                # b0's e-tiles (-> V kc8-15) only land ~G12-15, so block 0's
                # late ctx chunks shift into block 1's window; block 1 then
                # waits for block 0's pctx drains, block 2 half-recovers and
                # block 3+ runs the steady-state template.
                if bi == 0:
                    cg = [4, 6, 8, 10, 20, 22, 24, 26]
                    dr, nr, dm = 28, 30, 32
                elif bi == 1:
                    cg = [30, 31, 32, 33, 34, 35, 36, 37]
                    dr, nr, dm = 38, 40, 42
                elif bi == 2:
                    # j7 reads group 47's slots -> gate 48 (after exp(47))
                    cg = [40, 41, 42, 43, 44, 45, 46, 48]
                    dr, nr, dm = 49, 50, 52
                elif bi >= 4:
                    # the back half is ACT-bound with PE slack: shift ctx
                    # late to unload the front, tapering so each block's
                    # pctx drain still precedes the next block's alloc
                    s = 8 - bi  # 4,3,2,1 for blocks 4-7
                    cg = [base + 2 * j + 2 + s for j in range(8)]
                    dr, nr, dm = base + 17 + s, base + 18 + s, base + 20 + s
                else:
                    cg = [base + 2 * j + 2 for j in range(8)]
                    dr, nr, dm = base + 17, base + 18, base + 20
                for j in range(8):
                    if bi == NB - 1 and j == 7:
                        # split the final chunk so only one 1-kc matmul per
                        # head remains after the last exp group
                        for h in range(HPC):
                            tasks.append(T(base + 15, base + 19, 215, ctx_kc(h, 14)))
                        for h in range(HPC):
                            tasks.append(T(base + 16, base + 20, 215, ctx_kc(h, 15)))
                        continue
                    for h in range(HPC):
                        tasks.append(T(cg[j] + h, cg[j] + h + 4, 430, ctx_chunk(h, j)))
                if bi < NB - 1:
                    for h in range(HPC):
                        tasks.append(T(dr + h, dr + h + 4, 100, drain(h)))
                        tasks.append(T(nr + h, nr + h + 5, 600, norm(h)))
                    tasks.append(T(dm, dm + 6, 0, dma_out(0, TPB)))
                else:
                    # fine-grained tail: 2-t norm chunks and split out-DMAs
                    # so the post-exp critical chain is as short as possible
                    tasks.append(T(base + 17, base + 23, 100, drain(0)))
                    tasks.append(T(base + 18, base + 24, 300, norm(0, 0, 2, "tp")))
                    tasks.append(T(base + 18, base + 25, 100, drain(1)))
                    tasks.append(T(base + 19, base + 25, 300, norm(0, 2, TPB, "ps")))
                    tasks.append(T(base + 19, base + 26, 300, norm(1, 0, 2, "tp")))
                    tasks.append(T(base + 20, base + 26, 0, dma_out(0, 2)))
                    tasks.append(T(base + 20, base + 27, 300, norm(1, 2, TPB, "ps")))
                    tasks.append(T(base + 21, base + 27, 0, dma_out(2, TPB)))
                return tasks

            # ---------- global filler task list ----------
            fillers = []

            def qk_task(b, r, which, gate, dl):
                fillers.append(
                    T(gate, dl, 1800, lambda b=b, r=r, w=which: qk_proj(b, r, w, psmall, "ps"))
                )

            def v_task(b, r, half, gate, dl):
                fillers.append(
                    T(gate, dl, 1040, lambda b=b, r=r, hf=half: v_proj(b, r, hf))
                )

            # b0 prep: e r1-3 proj feed score kc 4..15 (group == kc); gates
            # match expected DMA arrival so pops don't block the PE queue
            qk_task(0, 1, "e", 0, 4)
            qk_task(0, 2, "e", 4, 8)
            qk_task(0, 3, "e", 8, 12)
            v_task(0, 0, 0, 0, 4)
            v_task(0, 0, 1, 1, 6)
            v_task(0, 1, 0, 4, 8)
            v_task(0, 1, 1, 5, 10)
            v_task(0, 2, 0, 8, 20)
            v_task(0, 2, 1, 9, 22)
            v_task(0, 3, 0, 12, 24)
            v_task(0, 3, 1, 13, 26)
            qk_task(0, 1, "q", 6, 16)
            qk_task(0, 2, "q", 18, 32)
            qk_task(0, 3, "q", 30, 48)

            # b1 source DMA issues (esrc/qsrc rings are deep enough that
            # slot reuse only needs b0-r-readers of the same slot emitted)
            def dma_b1(which, r, eng):
                def run():
                    dma_src(which, 1, r, 0, 4, eng)
                    dma_src(which, 1, r, 4, 8, eng)
                return run

            for r in range(RC_B):
                fillers.append(T(12 + r, 30 + 4 * r, 0, dma_b1("e", r, nc.sync)))
            # q-ring slot reuse: q1r2/q1r3 land on q02/q03 slots, whose
            # readers qproj(0,2)/(0,3) are emitted by G30/G46 (deadlines).
            for r, gate in enumerate((17, 18, 31, 47)):
                fillers.append(T(gate, 48 + 4 * r, 0, dma_b1("q", r, nc.gpsimd)))

            # b1 prep — gated as early as the b1 DMAs plausibly land, with
            # deadlines a few groups before the scores that need them so the
            # work spreads over blocks 2-3 instead of bunching at G62-79
            qk_task(1, 0, "e", 28, 54)
            qk_task(1, 1, "e", 32, 56)
            qk_task(1, 2, "e", 36, 58)
            qk_task(1, 3, "e", 40, 60)
            qk_task(1, 0, "q", 42, 60)
            # vproj(1) is only needed from G70 (shifted ctx(4)) — park it in
            # the ACT-bound back half instead of the oversubscribed front
            for r in range(RC_B):
                for half in (0, 1):
                    j = 2 * r + half
                    v_task(1, r, half, 56 + 2 * j, 70 + 2 * j)
            qk_task(1, 1, "q", 64, 78)
            qk_task(1, 2, "q", 72, 94)
            qk_task(1, 3, "q", 88, 110)

            for bi in range(NB):
                fillers += make_ctx_tasks(bi)

            fillers.sort(key=lambda t: (t.gate, t.deadline))
            total_cost = sum(t.cost for t in fillers)

            # ---------- startup projections (r0 of b0) ----------
            qk_proj(0, 0, "e", spp, "sp")
            qk_proj(0, 0, "q", spp, "sp")

            # ---------- flat attention stream ----------
            def scores_for(X):
                bi, g = divmod(X, NG)
                b, qc = ORDER[bi]
                col0 = qc * NFREE
                sp = spp.tile([P, HPC, NFREE], F32, tag="sp", name=f"sp{X}")
                for h in range(HPC):
                    d0 = h * HD
                    nc.tensor.matmul(
                        sp[:, h, :],
                        lhsT=kt_sb[b][d0 : d0 + HD, g * P : (g + 1) * P],
                        rhs=qt_sb[b][d0 : d0 + HD, col0 : col0 + NFREE],
                        start=True,
                        stop=True,
                    )
                return sp

            sps = {0: scores_for(0), 1: scores_for(1)}
            done_cost = 0.0
            for G in range(NGT):
                bi, g = divmod(G, NG)
                if g == 0:
                    pr_holder[bi] = prp.tile(
                        [P, NSLOT, NFREE], BF16, tag="pr", name=f"pr{bi}"
                    )
                # forced pops: anything that must precede scores(G+2)
                i = 0
                while i < len(fillers):
                    if fillers[i].deadline <= G + 2:
                        t = fillers.pop(i)
                        t.fn()
                        done_cost += t.cost
                    else:
                        i += 1
                # budget pops: gate-ready work, rate-limited so a burst of
                # ready tasks cannot starve the exp stream
                want = total_cost * (G + 1) / NGT
                while fillers and fillers[0].gate <= G and done_cost < want:
                    t = fillers.pop(0)
                    t.fn()
                    done_cost += t.cost
                if G + 2 < NGT:
                    sps[G + 2] = scores_for(G + 2)
                pr = pr_holder[bi]
                nc.scalar.activation(
                    pr[:, HPC * g : HPC * g + HPC, :], sps.pop(G), AF.Exp
                )
            while fillers:
                fillers.pop(0).fn()

    nc.finalize()
    return nc


def _get_nc(with_bias: bool = True) -> bass.Bass:
    if with_bias not in _CACHED_NC:
        _CACHED_NC[with_bias] = _build_nc(with_bias)
    return _CACHED_NC[with_bias]


def kernel(embed, q, Wk, bk, Wq, bq, Wv, bv, trace=False):
    global LAST_RESULTS
    bf = ml_dtypes.bfloat16
    embed = np.asarray(embed, dtype=np.float32)
    q = np.asarray(q, dtype=np.float32)
    Wk = np.asarray(Wk, dtype=np.float32)
    Wq = np.asarray(Wq, dtype=np.float32)
    Wv = np.asarray(Wv, dtype=np.float32)
    bk = np.asarray(bk, dtype=np.float32)
    bq = np.asarray(bq, dtype=np.float32)
    bv = np.asarray(bv, dtype=np.float32)

    qTt = _tile_inputs(q.reshape(ROWS, E))
    eTt = _tile_inputs(embed.reshape(ROWS, E))

    in_maps = []
    for c in range(NCORES):
        sl = slice(c * DPC, (c + 1) * DPC)
        in_maps.append(
            {
                "qTt": qTt,
                "eTt": eTt,
                # scores scale folded into Wq/bq (exact: *2^-3)
                "WqT": _tile_w(np.ascontiguousarray((Wq[sl] * SCALE).T).astype(bf)),
                "WkT": _tile_w(np.ascontiguousarray(Wk[sl].T).astype(bf)),
                "WvT": _tile_w(np.ascontiguousarray(Wv[sl].T).astype(bf)),
                "bqs": (bq[sl] * SCALE).astype(bf),
                "bkp": bk[sl].astype(bf),
                "bvp": bv[sl].astype(bf),
                "idn": np.eye(P, dtype=np.float32).astype(bf),
            }
        )

    with_bias = bool(bq.any() or bk.any() or bv.any())
    nc = _get_nc(with_bias)
    res = run_bass_kernel_spmd(nc, in_maps, list(range(NCORES)), trace=trace)
    LAST_RESULTS = res

    full = np.empty((ROWS, E), dtype=np.float32)
    for c in range(NCORES):
        full[:, c * DPC : (c + 1) * DPC] = res.results[c]["out"]
    return full.reshape(B, S, E)
